# revision 57
# baseline (speedup 1.0000x reference)
"""Trainium2 Bass kernel for GroupQueryAttention (T=1024, D=2048, H=32, KV=8, HD=64).

Sharding: tensor-parallel over heads across 8 NeuronCores. Core r owns q-heads
4r..4r+3 and kv-head r (split Wq/Wk/Wv output dims). x is replicated. The
per-core attention outputs are exchanged with an AllToAll (split in two
pieces, overlapped with compute) and the out-projection is token-parallel:
each core computes its 128-token slice of the final [1024, 2048] output with
the full 2048-deep contraction.

Per-core dataflow:
  A) fused QKV projection run k-outer over 8 concurrent psum groups so the
     PE paces with the x/w DMA stream (inputs split per 2-k-tile chunk over
     both HWDGE queues); then a software-pipelined RMSNorm + RoPE chain in
     bf16 on DVE/GpSimd (norm weight folded into host-side cos/sin tables,
     stats -> sqrt -> rope -> PE-transpose each lagged one t-tile).
  B) attention in transposed layout, heads processed in pairs (h even at
     partitions 0-63, h odd at 64-127) so the K=64 score matmuls of the two
     heads run concurrently in different PE row groups; AV matmuls lag the
     scores by one s-tile so the PE never queues behind an exp. The V tile
     carries 64 ones-columns, so the AV matmul emits O^T (psum rows 0-63)
     and the softmax denominator replicated 64x (rows 64-127) in one
     stream; the normalize tail (den copy, partition-hop DMA, approx
     reciprocal, multiply) runs per 512-col chunk -- the first half
     mid-pair, off the critical path.
  C) two-piece AllToAll (heads 0-1 staged right after pair 1 and overlapped
     with pair-2 compute; heads 2-3 after). The out-projection accumulates
     piece-1 k-tiles (even) while piece 2 is on the wire. Dep-free
     warm-keeper matmuls on scratch data bridge the PE-idle collective
     gaps so the out-projection runs at full clock (HAM stays engaged).
"""
import sys
import numpy as np
import ml_dtypes

sys.path.insert(0, "/opt/trn_rl_repo")

import concourse.bass as bass
import concourse.mybir as mybir
import concourse.tile as tile
import concourse.bacc as bacc
from concourse.bass_utils import run_bass_kernel_spmd
from concourse.masks import make_identity
from concourse.tile_rust import add_dep_helper
from contextlib import ExitStack

N_CORES = 8
T, D, H, KVH, HD = 1024, 2048, 32, 8, 64
GROUP = H // KVH          # 4 q-heads per kv-head
HQ = H // N_CORES         # 4 q-heads per core
E = HQ * HD               # 256 = per-core q width
QKV = E + HD + HD         # 384 = fused projection width
NT = T // 128             # 8 t-tiles
NK = D // 128             # 16 contraction tiles
EPS = 1e-6
F32 = mybir.dt.float32
BF16 = mybir.dt.bfloat16
TS = T // N_CORES         # 128 = per-core token slice of the output

_NC_CACHE = {}


def chunks_for(width):
    out = []
    for a in range(0, width, 512):
        out.append((a, min(width - a, 512)))
    return out


def build_nc(debug=False, reps=1, no_collective=False, phases="ABC",
             skip_in_dma=False, serial=False):
    key = ("nc", debug, reps, no_collective, phases, skip_in_dma, serial)
    if key in _NC_CACHE:
        return _NC_CACHE[key]
    nc = bacc.Bacc("TRN2", target_bir_lowering=False, debug=False,
                   num_devices=N_CORES)
    mul = mybir.AluOpType.mult
    add = mybir.AluOpType.add

    xT = nc.dram_tensor("xT", [D, T], BF16, kind="ExternalInput").ap()
    wqkv = nc.dram_tensor("wqkv", [D, QKV], BF16, kind="ExternalInput").ap()
    wo = nc.dram_tensor("wo", [D, D], BF16, kind="ExternalInput").ap()
    cos_all = nc.dram_tensor("cos_all", [T, E + HD], BF16,
                             kind="ExternalInput").ap()
    sin_all = nc.dram_tensor("sin_all", [T, E + HD], BF16,
                             kind="ExternalInput").ap()
    maskmul = nc.dram_tensor("maskmul", [128, NT, 128], BF16,
                             kind="ExternalInput").ap()
    outT = nc.dram_tensor("outT", [TS, D], F32, kind="ExternalOutput").ap()
    # sink for the PE warm-keeper chains (never read by the host)
    warm_sink = nc.dram_tensor("warm_sink", [1, 2], F32,
                               kind="ExternalOutput").ap()
    dbg = {}
    if debug:
        for nm, shape in [("d_qv", [128, NT, 448]), ("d_roped", [128, NT, E + HD]),
                          ("d_qT", [128, 2, T]), ("d_kT", [128, T]),
                          ("d_exp00", [128, T]), ("d_exp03", [128, T]),
                          ("d_pot0", [128, T]), ("d_rcp0", [64, T]),
                          ("d_ot0", [64, T])]:
            dbg[nm] = nc.dram_tensor(nm, shape, F32, kind="ExternalOutput").ap()

    hview = lambda ap: ap.rearrange("p (h d) -> p h d", h=HQ + 1)

    with tile.TileContext(nc) as tc:
        with ExitStack() as top:
            persist = top.enter_context(tc.tile_pool(name="persist", bufs=1))
            dram = top.enter_context(tc.tile_pool(name="dram", bufs=1, space="DRAM"))

            # ---- one-time constants ----
            ident = persist.tile([128, 128], BF16)
            make_identity(nc, ident[:])
            eps_t = persist.tile([128, 1], F32)
            nc.gpsimd.memset(eps_t[:], EPS)
            # scratch for the PE warm-keeper chains (values irrelevant)
            warm_w = persist.tile([128, 128], BF16)
            nc.gpsimd.memset(warm_w[:], 0.0)
            warm_x = persist.tile([128, 512], BF16)
            nc.gpsimd.memset(warm_x[:], 0.0)
            warm_wf = persist.tile([128, 128], F32)
            nc.gpsimd.memset(warm_wf[:], 0.0)
            warm_xf = persist.tile([128, 512], F32)
            nc.gpsimd.memset(warm_xf[:], 0.0)

            # ---- persistent (per-rep-rewritten) tiles ----
            qT_all = persist.tile([128, 2, T], BF16, name="qT_all")
            kT_all = persist.tile([128, T], BF16, name="kT_all")
            # per-t-tile fused [ones 0:64 | v 64:128 | qk 128:448]; the
            # AV lhsT is cols 0:128, so the softmax denominator comes out
            # at psum partitions 0-63 (base 0 -- custom-DVE-op legal) and
            # O^T at 64-127
            qv = [persist.tile([128, 448], BF16, name=f"qv{j}")
                  for j in range(NT)]
            cs_c = persist.tile([128, NT, E + HD], BF16, name="cs_c")
            cs_s = persist.tile([128, NT, E + HD], BF16, name="cs_s")
            mask_sb = persist.tile([128, NT, 128], BF16, name="mask_sb")
            wo_sb = persist.tile([128, NK, D], BF16, name="wo_sb")
            xt_g = [persist.tile([128, 2, T], BF16, name=f"xt{g}")
                    for g in range(NK // 2)]
            wq_g = [persist.tile([128, 2, QKV], BF16, name=f"wq{g}")
                    for g in range(NK // 2)]

            xT_v = xT.rearrange("(k p) t -> p k t", p=128)
            wq_v = wqkv.rearrange("(k p) n -> p k n", p=128)
            wo_v = wo.rearrange("(k p) n -> p k n", p=128)

            cos_v = cos_all.rearrange("(j p) n -> p j n", p=128)
            sin_v = sin_all.rearrange("(j p) n -> p j n", p=128)

            def load_inputs():
                ins = []
                # rope tables for the first two t-tiles up front (tiny);
                # the rest after x/w so the projection isn't DMA-starved
                ins.append(nc.scalar.dma_start(cs_c[:, 0:2, :],
                                               cos_v[:, 0:2, :]))
                ins.append(nc.scalar.dma_start(cs_s[:, 0:2, :],
                                               sin_v[:, 0:2, :]))
                # x^T and W_qkv^T split per 2-k-tile group, alternating the
                # two HWDGE queues so the first QKV matmul starts early
                for g in range(NK // 2):
                    eng = nc.sync if g % 2 == 0 else nc.scalar
                    ins.append(eng.dma_start(xt_g[g][:], xT_v[:, 2 * g:2 * g + 2, :]))
                    ins.append(eng.dma_start(wq_g[g][:], wq_v[:, 2 * g:2 * g + 2, :]))
                ins.append(nc.scalar.dma_start(cs_c[:, 2:NT, :],
                                               cos_v[:, 2:NT, :]))
                ins.append(nc.scalar.dma_start(cs_s[:, 2:NT, :],
                                               sin_v[:, 2:NT, :]))
                ins.append(nc.scalar.dma_start(mask_sb[:], maskmul[:]))
                # 8MB Wo load in the background, after x/w on both queues
                for q in range(8):
                    eng = nc.sync if q % 2 == 0 else nc.scalar
                    ins.append(eng.dma_start(
                        wo_sb[:, 2 * q:2 * (q + 1), :],
                        wo_v[:, 2 * q:2 * (q + 1), :]))
                return ins

            prev_tail = None
            for _rep in range(reps):
                entries = load_inputs()
                # ones columns for the AV denominator rows
                for j in range(NT):
                    nc.gpsimd.memset(qv[j][:, 0:64], 1.0)

                # ---- phase A: QKV projection + norm + rope + transposes ----
                pend_tr = []  # transposes lagged one t-tile to keep PE dense

                def do_transposes(j, roped):
                    for m in range(2):
                        ptr = ps_tr.tile([128, 128], BF16, name="ptr", tag="ptr")
                        nc.tensor.transpose(
                            ptr[:], roped[:, 128 * m:128 * (m + 1)], ident[:])
                        nc.scalar.copy(qT_all[:, m, 128 * j:128 * (j + 1)], ptr[:])
                    ptrk = ps_tr.tile([64, 128], BF16, name="ptrk", tag="ptrk")
                    nc.tensor.transpose(ptrk[:], roped[:, E:E + HD], ident[:])
                    nc.scalar.copy(kT_all[0:HD, 128 * j:128 * (j + 1)], ptrk[:])
                    # per-tile dup into partitions 64..127 for odd-head MMs
                    return nc.sync.dma_start(
                        kT_all[HD:128, 128 * j:128 * (j + 1)],
                        kT_all[0:HD, 128 * j:128 * (j + 1)])

                # A-1: QKV projection, k-outer over 8 concurrent psum
                # groups -- each matmul needs only contraction chunk k, so
                # the PE paces with the x/w DMA stream instead of
                # head-of-line blocking on the last chunk.
                with ExitStack() as pa0:
                    ps_qkv = pa0.enter_context(
                        tc.tile_pool(name="ps_qkv", bufs=1, space="PSUM"))
                    pqs = [ps_qkv.tile([128, QKV], F32, name=f"pq{j}",
                                       tag=f"pq{j}") for j in range(NT)]
                    for k in range(NK):
                        for j in range(NT):
                            nc.tensor.matmul(
                                pqs[j][:],
                                xt_g[k // 2][:, k % 2, 128 * j:128 * (j + 1)],
                                wq_g[k // 2][:, k % 2, :],
                                start=(k == 0), stop=(k == NK - 1))
                    for j in range(NT):
                        # single psum eviction: Q|K for rope + V slice
                        nc.scalar.copy(qv[j][:, 64:448], pqs[j][:])
                        if debug:
                            nc.sync.dma_start(dbg["d_qv"][:, j, :],
                                              qv[j][:].bitcast(F32))

                # A-2: norm + rope + transposes (psum freed by A-1)
                with ExitStack() as pa:
                    scrA = pa.enter_context(tc.tile_pool(name="scrA", bufs=4))
                    ps_tr = pa.enter_context(
                        tc.tile_pool(name="ps_tr", bufs=2, space="PSUM"))

                    stats = {}

                    def do_stats(j):
                        # sum of squares per head for t-tile j (DVE)
                        qk = qv[j][:, 128:448]
                        sq = scrA.tile([128, E + HD], BF16, name="sq", tag="sq")
                        nc.vector.tensor_tensor(out=sq[:], in0=qk, in1=qk, op=mul)
                        ssq = scrA.tile([128, 8], F32, name="ssq", tag="ssq")
                        nc.vector.tensor_reduce(
                            out=ssq[:, 0:HQ + 1],
                            in_=sq[:].rearrange("p (h d) -> p h d", h=HQ + 1),
                            axis=mybir.AxisListType.X, op=add)
                        stats[j] = ssq

                    rmss = {}

                    def do_sqrt(j):
                        ssq = stats.pop(j)
                        rms = scrA.tile([128, 8], F32, name="rms", tag="rms")
                        nc.scalar.activation(rms[:, 0:HQ + 1], ssq[:, 0:HQ + 1],
                                             mybir.ActivationFunctionType.Sqrt,
                                             scale=1.0 / HD, bias=eps_t[:])
                        rmss[j] = rms

                    def do_rope(j):
                        # inv scale then rope (DVE/Pool)
                        qk = qv[j][:, 128:448]
                        rms = rmss.pop(j)
                        inv = scrA.tile([128, 8], F32, name="inv", tag="inv")
                        nc.vector.reciprocal(inv[:, 0:HQ + 1], rms[:, 0:HQ + 1])
                        # rope: qs = qk * inv[head]; m1 = qs*cos (gpsimd);
                        # m2 = swap(qs)*sin (gpsimd); roped = m1 + m2
                        qs = scrA.tile([128, E + HD], BF16, name="qs", tag="qs")
                        m1 = scrA.tile([128, E + HD], BF16, name="m1", tag="m1")
                        m2 = scrA.tile([128, E + HD], BF16, name="m2", tag="m2")
                        roped = scrA.tile([128, E + HD], BF16, name="roped",
                                          tag="roped")
                        inv_b = (inv[:, 0:HQ + 1]
                                 .rearrange("p (h o) -> p h o", o=1)
                                 .broadcast_to([128, HQ + 1, HD]))
                        nc.vector.tensor_tensor(out=hview(qs[:]),
                                                in0=hview(qk),
                                                in1=inv_b, op=mul)
                        nc.vector.tensor_tensor(
                            out=hview(m1[:]), in0=hview(qs[:]),
                            in1=hview(cs_c[:, j, :]), op=mul)
                        nc.gpsimd.tensor_tensor(
                            out=hview(m2[:])[:, :, 0:32],
                            in0=hview(qs[:])[:, :, 32:HD],
                            in1=hview(cs_s[:, j, :])[:, :, 0:32], op=mul)
                        nc.gpsimd.tensor_tensor(
                            out=hview(m2[:])[:, :, 32:HD],
                            in0=hview(qs[:])[:, :, 0:32],
                            in1=hview(cs_s[:, j, :])[:, :, 32:HD], op=mul)
                        nc.vector.tensor_tensor(out=roped[:], in0=m1[:],
                                                in1=m2[:], op=add)
                        if debug:
                            nc.sync.dma_start(dbg["d_roped"][:, j, :],
                                              roped[:].bitcast(F32))
                        pend_tr.append((j, roped))

                    # software pipeline: sqrt lags stats by one tile, rope
                    # by two, transposes by three, so no engine queue blocks
                    # on work another engine hasn't finished yet
                    for j in range(NT):
                        do_stats(j)
                        if j >= 1:
                            do_sqrt(j - 1)
                        if j >= 2:
                            do_rope(j - 2)
                        if j >= 3:
                            do_transposes(*pend_tr.pop(0))
                    do_sqrt(NT - 1)
                    do_rope(NT - 2)
                    do_transposes(*pend_tr.pop(0))
                    do_rope(NT - 1)
                    while pend_tr:
                        tail = do_transposes(*pend_tr.pop(0))

                def fin():
                    nonlocal prev_tail
                    if serial and prev_tail is not None:
                        for e in entries:
                            add_dep_helper(e.ins, prev_tail.ins, sync=True,
                                           reason="serialize reps")
                    prev_tail = tail
                if debug:
                    nc.sync.dma_start(dbg["d_qT"][:], qT_all[:].bitcast(F32))
                    nc.sync.dma_start(dbg["d_kT"][:], kT_all[:].bitcast(F32))

                if "B" not in phases:
                    fin()
                    continue

                # ---- phase B: attention, head pairs in PE row groups ----
                a2a_in = [dram.tile([N_CORES, 2 * HD, TS], BF16,
                                    name=f"a2i{p}_{_rep}", tag=f"a2i{p}_{_rep}")
                          for p in range(2)]
                a2a_out = [dram.tile([N_CORES, 2 * HD, TS], BF16,
                                     name=f"a2o{p}_{_rep}", tag=f"a2o{p}_{_rep}")
                           for p in range(2)]
                colls = []
                with ExitStack() as pb:
                    epool = pb.enter_context(tc.tile_pool(name="expp", bufs=4))
                    bscr = pb.enter_context(tc.tile_pool(name="scrB", bufs=2))
                    ps_st = pb.enter_context(
                        tc.tile_pool(name="ps_st", bufs=4, space="PSUM"))
                    ps_ot = pb.enter_context(
                        tc.tile_pool(name="ps_ot", bufs=2, space="PSUM"))

                    for p in range(2):
                        pots = [ps_ot.tile([128, T], F32, name=f"pot{p}{hh}",
                                           tag="pot") for hh in range(2)]
                        ots = [bscr.tile([128, T], BF16, name=f"ot{hh}",
                                         tag="ot") for hh in range(2)]

                        def do_tail(a, b):
                            # normalize + stage O^T for psum cols [a, b):
                            # rcp of den (psum-direct at base 0) -> replicate
                            # to partitions 64-127 via identity matmul -> mul
                            wdt = b - a
                            for hh in range(2):
                                rcp = bscr.tile([HD, 512], F32,
                                                name="rcp", tag="rcp")
                                nc.vector.reciprocal_approx_fast(
                                    out=rcp[:, 0:wdt], in_=pots[hh][0:HD, a:b])
                                rcp_hi = bscr.tile([128, 512], F32,
                                                   name="rcphi", tag="rcphi")
                                nc.scalar.dma_start(rcp_hi[HD:128, 0:wdt],
                                                  rcp[:, 0:wdt])
                                nc.vector.tensor_tensor(
                                    out=ots[hh][HD:128, a:b],
                                    in0=pots[hh][HD:128, a:b],
                                    in1=rcp_hi[HD:128, 0:wdt], op=mul)
                                j0, j1 = a // TS, b // TS
                                nc.sync.dma_start(
                                    a2a_in[p][j0:j1, HD * hh:HD * (hh + 1), :]
                                    .rearrange("j d t -> d j t"),
                                    ots[hh][HD:128, a:b].rearrange(
                                        "d (j t) -> d j t", j=j1 - j0))

                        def do_av(i, ets):
                            t0 = 128 * i
                            for hh in range(2):
                                for (a, b) in ((0, 512), (512, 1024)):
                                    if t0 >= b:
                                        continue
                                    lo = max(a, t0)
                                    nc.tensor.matmul(
                                        pots[hh][:, lo:b],
                                        qv[i][:, 0:128],
                                        ets[hh][:, lo - t0:b - t0],
                                        start=(i == 0),
                                        stop=(i == min(b // 128, NT) - 1))

                        # AV matmuls lag the score matmuls by one s-tile so
                        # the PE never queues behind an exp it must wait for
                        pend_av = []
                        for i in range(NT):
                            t0 = 128 * i
                            w = T - t0
                            ets = [epool.tile([128, T], BF16, name=f"et{hh}",
                                              tag="et") for hh in range(2)]
                            # paired score matmuls in different PE row
                            # groups, 512-col psum chunks (1 bank each) so
                            # the pipeline holds 4 outstanding chunks
                            for (c0, cw) in chunks_for(w):
                                psts = [ps_st.tile([128, 512], F32,
                                                   name="pst", tag="pst")
                                        for hh in range(2)]
                                for hh in range(2):
                                    p0 = 64 * hh
                                    nc.tensor.matmul(
                                        psts[hh][:, 0:cw],
                                        kT_all[p0:p0 + HD, t0:t0 + 128],
                                        qT_all[p0:p0 + HD, p,
                                               t0 + c0:t0 + c0 + cw],
                                        start=True, stop=True)
                                for hh in range(2):
                                    nc.scalar.activation(
                                        ets[hh][:, c0:c0 + cw],
                                        psts[hh][:, 0:cw],
                                        mybir.ActivationFunctionType.Exp,
                                        scale=float(1.0 / np.sqrt(HD)))
                                    if c0 == 0:
                                        # causal mask on the diagonal block
                                        meng = nc.gpsimd if p == 0 else nc.vector
                                        meng.tensor_tensor(
                                            out=ets[hh][:, 0:128],
                                            in0=ets[hh][:, 0:128],
                                            in1=mask_sb[:, i, :], op=mul)
                            pend_av.append((i, ets))
                            if len(pend_av) > 1:
                                do_av(*pend_av.pop(0))
                                if i == 4:
                                    # psum cols 0-511 complete after AV(3):
                                    # run half the normalize tail mid-pair
                                    do_tail(0, 512)
                        do_av(*pend_av.pop(0))
                        do_tail(512, 1024)
                        if debug and p == 0:
                            dpot = bscr.tile([128, T], F32, name="dpot", tag="dpot")
                            nc.vector.tensor_copy(dpot[:], pots[0][:])
                            nc.sync.dma_start(dbg["d_pot0"][:], dpot[:])
                            nc.sync.dma_start(dbg["d_ot0"][:],
                                              ots[0][HD:128, :].bitcast(F32))
                        if no_collective:
                            colls.append(nc.sync.dma_start(a2a_out[p][:],
                                                           a2a_in[p][:]))
                        else:
                            colls.append(nc.gpsimd.collective_compute(
                                "AllToAll", mybir.AluOpType.bypass,
                                replica_groups=[list(range(N_CORES))],
                                ins=[a2a_in[p].opt()], outs=[a2a_out[p].opt()]))
                tail = colls[-1]

                # ---- phase C: token-parallel out-projection ----
                # contraction row 256*src + 64*h + d: piece 0 (heads 0-1)
                # supplies even k-tiles, piece 1 (heads 2-3) odd k-tiles.
                if "C" not in phases:
                    fin()
                    continue
                with ExitStack() as pc:
                    agp = pc.enter_context(tc.tile_pool(name="agp", bufs=1))
                    oscr = pc.enter_context(tc.tile_pool(name="oscr", bufs=1))
                    ps_o = pc.enter_context(
                        tc.tile_pool(name="ps_o", bufs=1, space="PSUM"))
                    ps_w = pc.enter_context(
                        tc.tile_pool(name="ps_w", bufs=1, space="PSUM"))

                    def keep_warm(n, idx, fp32):
                        # dep-free matmuls bridge the PE-idle gap of an
                        # in-flight collective so the next real matmuls run
                        # at full clock (HAM stays engaged); sunk to DRAM so
                        # the chain can't be pruned
                        pw = ps_w.tile([128, 512], F32, name=f"pw{idx}",
                                       tag="pw")
                        ww, wx = (warm_wf, warm_xf) if fp32 else (warm_w, warm_x)
                        for k in range(n):
                            nc.tensor.matmul(pw[:], ww[:], wx[:],
                                             start=(k == 0), stop=(k == n - 1))
                        wsb = oscr.tile([1, 2], F32, name=f"wsb{idx}",
                                        tag=f"wsb{idx}")
                        nc.scalar.copy(wsb[:], pw[0:1, 0:2])
                        nc.sync.dma_start(warm_sink[:], wsb[:])

                    pos = [ps_o.tile([128, 512], F32, name=f"po{c}",
                                     tag=f"po{c}") for c in range(4)]
                    for p in range(2):
                        keep_warm(*((12, 0, False) if p == 0 else (16, 1, True)))
                        agt = agp.tile([128, NT, TS], BF16, name=f"agt{p}",
                                       tag=f"agt{p}")
                        nc.scalar.dma_start(
                            agt[:], a2a_out[p].rearrange("s p t -> p s t"))
                        for s in range(NT):
                            for c in range(4):
                                nc.tensor.matmul(
                                    pos[c],
                                    agt[:, s, :],
                                    wo_sb[:, 2 * s + p, 512 * c:512 * (c + 1)],
                                    start=(p == 0 and s == 0),
                                    stop=(p == 1 and s == NT - 1))
                    out_sb = oscr.tile([128, 4, 512], F32, name="out_sb",
                                       tag="out_sb")
                    for c in range(4):
                        if c % 2 == 0:
                            nc.scalar.copy(out_sb[:, c, :], pos[c][:])
                        else:
                            nc.vector.tensor_copy(out_sb[:, c, :], pos[c][:])
                        tail = nc.sync.dma_start(
                            outT[:, 512 * c:512 * (c + 1)], out_sb[:, c, :])
                fin()

    nc.compile()
    _NC_CACHE[key] = nc
    return nc


def prep_in_maps(inputs):
    """Host-side sharding + layout prep. Returns per-core input maps."""
    x = np.asarray(inputs["x"], dtype=np.float32)
    mask = np.asarray(inputs["mask"])
    cos = np.asarray(inputs["cos"], dtype=np.float32)
    sin = np.asarray(inputs["sin"], dtype=np.float32)
    Wq = np.asarray(inputs["Wq"], dtype=np.float32)
    Wk = np.asarray(inputs["Wk"], dtype=np.float32)
    Wv = np.asarray(inputs["Wv"], dtype=np.float32)
    Wo = np.asarray(inputs["Wo"], dtype=np.float32)
    qw = np.asarray(inputs["q_norm_w"], dtype=np.float32)
    kw = np.asarray(inputs["k_norm_w"], dtype=np.float32)

    bf = ml_dtypes.bfloat16
    xT = np.ascontiguousarray(x.T).astype(bf)

    # norm weights folded into per-(q/k) rope tables; sin table carries the
    # rotate-half signs: out[i] = w[i]*(q[i]*cos[i] + rot[i]*sin[i]),
    # rot[i] = -q[i+32] (i<32) else q[i-32]
    sgn = np.concatenate([-np.ones(HD // 2, np.float32),
                          np.ones(HD // 2, np.float32)])
    cos_q = cos * qw[None, :]
    sin_q = sin * (sgn * qw)[None, :]
    cos_k = cos * kw[None, :]
    sin_k = sin * (sgn * kw)[None, :]
    cos_all = np.ascontiguousarray(
        np.concatenate([cos_q] * HQ + [cos_k], axis=1)).astype(bf)
    sin_all = np.ascontiguousarray(
        np.concatenate([sin_q] * HQ + [sin_k], axis=1)).astype(bf)

    # multiplicative mask for diagonal blocks, in S^T layout:
    # maskmul[s', i, t'] = 0 where mask[128i+t', 128i+s'] else 1
    mm = np.ones((128, NT, 128), np.float32)
    for i in range(NT):
        blk = mask[128 * i:128 * (i + 1), 128 * i:128 * (i + 1)]
        mm[:, i, :] = np.where(blk.T, 0.0, 1.0).astype(np.float32)
    maskmul = mm.astype(bf)

    # token-parallel out-projection: full Wo^T on every core; its
    # contraction row order 256*src + 64*h + d matches the AllToAll output
    woT = np.ascontiguousarray(Wo.T).astype(bf)                # [2048, 2048]
    in_maps = []
    for r in range(N_CORES):
        wq_r = Wq[E * r:E * (r + 1), :]          # [256, 2048]
        wk_r = Wk[HD * r:HD * (r + 1), :]        # [64, 2048]
        wv_r = Wv[HD * r:HD * (r + 1), :]        # [64, 2048]
        # V first: the projection psum is [V | Q | K], evicted to qv cols
        # 64:448 behind the ones block
        wqkv = np.ascontiguousarray(
            np.concatenate([wv_r, wq_r, wk_r], axis=0).T).astype(bf)
        in_maps.append({
            "xT": xT, "wqkv": wqkv, "wo": woT,
            "cos_all": cos_all, "sin_all": sin_all,
            "maskmul": maskmul,
        })
    return in_maps


def kernel(**inputs) -> np.ndarray:
    nc = build_nc()
    in_maps = prep_in_maps(inputs)
    res = run_bass_kernel_spmd(nc, in_maps, list(range(N_CORES)))
    out = np.empty((T, D), dtype=np.float32)
    for r in range(N_CORES):
        out[TS * r:TS * (r + 1), :] = res.results[r]["outT"]
    return out


# revision 64
# speedup vs baseline: 1.0003x; 1.0003x over previous
"""Trainium2 Bass kernel for GroupQueryAttention (T=1024, D=2048, H=32, KV=8, HD=64).

Sharding: tensor-parallel over heads across 8 NeuronCores. Core r owns q-heads
4r..4r+3 and kv-head r (split Wq/Wk/Wv output dims). x is replicated. The
per-core attention outputs are exchanged with an AllToAll (split in two
pieces, overlapped with compute) and the out-projection is token-parallel:
each core computes its 128-token slice of the final [1024, 2048] output with
the full 2048-deep contraction.

Per-core dataflow:
  A) fused QKV projection run k-outer over 8 concurrent psum groups so the
     PE paces with the x/w DMA stream (inputs split per 2-k-tile chunk over
     both HWDGE queues); then a software-pipelined RMSNorm + RoPE chain in
     bf16 on DVE/GpSimd (norm weight folded into host-side cos/sin tables,
     stats -> sqrt -> rope -> PE-transpose each lagged one t-tile).
  B) attention in transposed layout, heads processed in pairs (h even at
     partitions 0-63, h odd at 64-127) so the K=64 score matmuls of the two
     heads run concurrently in different PE row groups; AV matmuls lag the
     scores by one s-tile so the PE never queues behind an exp. The V tile
     carries 64 ones-columns, so the AV matmul emits O^T (psum rows 0-63)
     and the softmax denominator replicated 64x (rows 64-127) in one
     stream; the normalize tail (den copy, partition-hop DMA, approx
     reciprocal, multiply) runs per 512-col chunk -- the first half
     mid-pair, off the critical path.
  C) two-piece AllToAll (heads 0-1 staged right after pair 1 and overlapped
     with pair-2 compute; heads 2-3 after). The out-projection accumulates
     piece-1 k-tiles (even) while piece 2 is on the wire. Dep-free
     warm-keeper matmuls on scratch data bridge the PE-idle collective
     gaps so the out-projection runs at full clock (HAM stays engaged).
"""
import sys
import numpy as np
import ml_dtypes

sys.path.insert(0, "/opt/trn_rl_repo")

import concourse.bass as bass
import concourse.mybir as mybir
import concourse.tile as tile
import concourse.bacc as bacc
from concourse.bass_utils import run_bass_kernel_spmd
from concourse.masks import make_identity
from concourse.tile_rust import add_dep_helper
from contextlib import ExitStack

N_CORES = 8
T, D, H, KVH, HD = 1024, 2048, 32, 8, 64
GROUP = H // KVH          # 4 q-heads per kv-head
HQ = H // N_CORES         # 4 q-heads per core
E = HQ * HD               # 256 = per-core q width
QKV = E + HD + HD         # 384 = fused projection width
NT = T // 128             # 8 t-tiles
NK = D // 128             # 16 contraction tiles
EPS = 1e-6
F32 = mybir.dt.float32
BF16 = mybir.dt.bfloat16
TS = T // N_CORES         # 128 = per-core token slice of the output

_NC_CACHE = {}


def chunks_for(width):
    out = []
    for a in range(0, width, 512):
        out.append((a, min(width - a, 512)))
    return out


def build_nc(debug=False, reps=1, no_collective=False, phases="ABC",
             skip_in_dma=False, serial=False):
    key = ("nc", debug, reps, no_collective, phases, skip_in_dma, serial)
    if key in _NC_CACHE:
        return _NC_CACHE[key]
    nc = bacc.Bacc("TRN2", target_bir_lowering=False, debug=False,
                   num_devices=N_CORES)
    mul = mybir.AluOpType.mult
    add = mybir.AluOpType.add

    xT = nc.dram_tensor("xT", [D, T], BF16, kind="ExternalInput").ap()
    wqkv = nc.dram_tensor("wqkv", [D, QKV], BF16, kind="ExternalInput").ap()
    wo = nc.dram_tensor("wo", [D, D], BF16, kind="ExternalInput").ap()
    cos_all = nc.dram_tensor("cos_all", [T, E + HD], BF16,
                             kind="ExternalInput").ap()
    sin_all = nc.dram_tensor("sin_all", [T, E + HD], BF16,
                             kind="ExternalInput").ap()
    maskmul = nc.dram_tensor("maskmul", [128, NT, 128], BF16,
                             kind="ExternalInput").ap()
    outT = nc.dram_tensor("outT", [TS, D], F32, kind="ExternalOutput").ap()
    # sink for the PE warm-keeper chains (never read by the host)
    warm_sink = nc.dram_tensor("warm_sink", [1, 2], F32,
                               kind="ExternalOutput").ap()
    dbg = {}
    if debug:
        for nm, shape in [("d_qv", [128, NT, 448]), ("d_roped", [128, NT, E + HD]),
                          ("d_qT", [128, 2, T]), ("d_kT", [128, T]),
                          ("d_exp00", [128, T]), ("d_exp03", [128, T]),
                          ("d_pot0", [128, T]), ("d_rcp0", [64, T]),
                          ("d_ot0", [64, T])]:
            dbg[nm] = nc.dram_tensor(nm, shape, F32, kind="ExternalOutput").ap()

    hview = lambda ap: ap.rearrange("p (h d) -> p h d", h=HQ + 1)

    with tile.TileContext(nc) as tc:
        with ExitStack() as top:
            persist = top.enter_context(tc.tile_pool(name="persist", bufs=1))
            dram = top.enter_context(tc.tile_pool(name="dram", bufs=1, space="DRAM"))

            # ---- one-time constants ----
            ident = persist.tile([128, 128], BF16)
            make_identity(nc, ident[:])
            eps_t = persist.tile([128, 1], F32)
            nc.gpsimd.memset(eps_t[:], EPS)
            # scratch for the PE warm-keeper chains (values irrelevant)
            warm_w = persist.tile([128, 128], BF16)
            nc.gpsimd.memset(warm_w[:], 0.0)
            warm_x = persist.tile([128, 512], BF16)
            nc.gpsimd.memset(warm_x[:], 0.0)
            warm_wf = persist.tile([128, 128], F32)
            nc.gpsimd.memset(warm_wf[:], 0.0)
            warm_xf = persist.tile([128, 512], F32)
            nc.gpsimd.memset(warm_xf[:], 0.0)

            # ---- persistent (per-rep-rewritten) tiles ----
            qT_all = persist.tile([128, 2, T], BF16, name="qT_all")
            kT_all = persist.tile([128, T], BF16, name="kT_all")
            # per-t-tile fused [ones 0:64 | v 64:128 | qk 128:448]; the
            # AV lhsT is cols 0:128, so the softmax denominator comes out
            # at psum partitions 0-63 (base 0 -- custom-DVE-op legal) and
            # O^T at 64-127
            qv = [persist.tile([128, 448], BF16, name=f"qv{j}")
                  for j in range(NT)]
            cs_c = persist.tile([128, NT, E + HD], BF16, name="cs_c")
            cs_s = persist.tile([128, NT, E + HD], BF16, name="cs_s")
            mask_sb = persist.tile([128, NT, 128], BF16, name="mask_sb")
            wo_sb = persist.tile([128, NK, D], BF16, name="wo_sb")
            xt_g = [persist.tile([128, 2, T], BF16, name=f"xt{g}")
                    for g in range(NK // 2)]
            wq_g = [persist.tile([128, 2, QKV], BF16, name=f"wq{g}")
                    for g in range(NK // 2)]

            xT_v = xT.rearrange("(k p) t -> p k t", p=128)
            wq_v = wqkv.rearrange("(k p) n -> p k n", p=128)
            wo_v = wo.rearrange("(k p) n -> p k n", p=128)

            cos_v = cos_all.rearrange("(j p) n -> p j n", p=128)
            sin_v = sin_all.rearrange("(j p) n -> p j n", p=128)

            def load_inputs():
                ins = []
                # rope tables for the first two t-tiles up front (tiny);
                # the rest after x/w so the projection isn't DMA-starved
                ins.append(nc.scalar.dma_start(cs_c[:, 0:2, :],
                                               cos_v[:, 0:2, :]))
                ins.append(nc.scalar.dma_start(cs_s[:, 0:2, :],
                                               sin_v[:, 0:2, :]))
                # x^T and W_qkv^T split per 2-k-tile group, alternating the
                # two HWDGE queues so the first QKV matmul starts early
                for g in range(NK // 2):
                    eng = nc.sync if g % 2 == 0 else nc.scalar
                    ins.append(eng.dma_start(xt_g[g][:], xT_v[:, 2 * g:2 * g + 2, :]))
                    ins.append(eng.dma_start(wq_g[g][:], wq_v[:, 2 * g:2 * g + 2, :]))
                ins.append(nc.scalar.dma_start(cs_c[:, 2:NT, :],
                                               cos_v[:, 2:NT, :]))
                ins.append(nc.scalar.dma_start(cs_s[:, 2:NT, :],
                                               sin_v[:, 2:NT, :]))
                ins.append(nc.scalar.dma_start(mask_sb[:], maskmul[:]))
                # 8MB Wo load in the background, after x/w on both queues
                for q in range(8):
                    eng = nc.sync if q % 2 == 0 else nc.scalar
                    ins.append(eng.dma_start(
                        wo_sb[:, 2 * q:2 * (q + 1), :],
                        wo_v[:, 2 * q:2 * (q + 1), :]))
                return ins

            prev_tail = None
            for _rep in range(reps):
                entries = load_inputs()
                # ones columns for the AV denominator rows
                for j in range(NT):
                    nc.gpsimd.memset(qv[j][:, 0:64], 1.0)

                # ---- phase A: QKV projection + norm + rope + transposes ----
                pend_tr = []  # transposes lagged one t-tile to keep PE dense

                def do_transposes(j, roped):
                    for m in range(2):
                        ptr = ps_tr.tile([128, 128], BF16, name="ptr", tag="ptr")
                        nc.tensor.transpose(
                            ptr[:], roped[:, 128 * m:128 * (m + 1)], ident[:])
                        nc.scalar.copy(qT_all[:, m, 128 * j:128 * (j + 1)], ptr[:])
                    ptrk = ps_tr.tile([64, 128], BF16, name="ptrk", tag="ptrk")
                    nc.tensor.transpose(ptrk[:], roped[:, E:E + HD], ident[:])
                    nc.scalar.copy(kT_all[0:HD, 128 * j:128 * (j + 1)], ptrk[:])
                    # per-tile dup into partitions 64..127 for odd-head MMs
                    return nc.sync.dma_start(
                        kT_all[HD:128, 128 * j:128 * (j + 1)],
                        kT_all[0:HD, 128 * j:128 * (j + 1)])

                # A-1: QKV projection, k-outer over 8 concurrent psum
                # groups -- each matmul needs only contraction chunk k, so
                # the PE paces with the x/w DMA stream instead of
                # head-of-line blocking on the last chunk.
                with ExitStack() as pa0:
                    ps_qkv = pa0.enter_context(
                        tc.tile_pool(name="ps_qkv", bufs=1, space="PSUM"))
                    pqs = [ps_qkv.tile([128, QKV], F32, name=f"pq{j}",
                                       tag=f"pq{j}") for j in range(NT)]
                    for k in range(NK):
                        for j in range(NT):
                            nc.tensor.matmul(
                                pqs[j][:],
                                xt_g[k // 2][:, k % 2, 128 * j:128 * (j + 1)],
                                wq_g[k // 2][:, k % 2, :],
                                start=(k == 0), stop=(k == NK - 1))
                    for j in range(NT):
                        # single psum eviction: Q|K for rope + V slice
                        nc.scalar.copy(qv[j][:, 64:448], pqs[j][:])
                        if debug:
                            nc.sync.dma_start(dbg["d_qv"][:, j, :],
                                              qv[j][:].bitcast(F32))

                # A-2: norm + rope + transposes (psum freed by A-1)
                with ExitStack() as pa:
                    scrA = pa.enter_context(tc.tile_pool(name="scrA", bufs=4))
                    ps_tr = pa.enter_context(
                        tc.tile_pool(name="ps_tr", bufs=2, space="PSUM"))

                    stats = {}

                    def do_stats(j):
                        # sum of squares per head for t-tile j (DVE)
                        qk = qv[j][:, 128:448]
                        sq = scrA.tile([128, E + HD], BF16, name="sq", tag="sq")
                        nc.vector.tensor_tensor(out=sq[:], in0=qk, in1=qk, op=mul)
                        ssq = scrA.tile([128, 8], F32, name="ssq", tag="ssq")
                        nc.vector.tensor_reduce(
                            out=ssq[:, 0:HQ + 1],
                            in_=sq[:].rearrange("p (h d) -> p h d", h=HQ + 1),
                            axis=mybir.AxisListType.X, op=add)
                        stats[j] = ssq

                    rmss = {}

                    def do_sqrt(j):
                        ssq = stats.pop(j)
                        rms = scrA.tile([128, 8], F32, name="rms", tag="rms")
                        nc.scalar.activation(rms[:, 0:HQ + 1], ssq[:, 0:HQ + 1],
                                             mybir.ActivationFunctionType.Sqrt,
                                             scale=1.0 / HD, bias=eps_t[:])
                        rmss[j] = rms

                    def do_rope(j):
                        # inv scale then rope (DVE/Pool)
                        qk = qv[j][:, 128:448]
                        rms = rmss.pop(j)
                        inv = scrA.tile([128, 8], F32, name="inv", tag="inv")
                        nc.vector.reciprocal(inv[:, 0:HQ + 1], rms[:, 0:HQ + 1])
                        # rope: qs = qk * inv[head]; m1 = qs*cos (gpsimd);
                        # m2 = swap(qs)*sin (gpsimd); roped = m1 + m2
                        qs = scrA.tile([128, E + HD], BF16, name="qs", tag="qs")
                        m1 = scrA.tile([128, E + HD], BF16, name="m1", tag="m1")
                        m2 = scrA.tile([128, E + HD], BF16, name="m2", tag="m2")
                        roped = scrA.tile([128, E + HD], BF16, name="roped",
                                          tag="roped")
                        inv_b = (inv[:, 0:HQ + 1]
                                 .rearrange("p (h o) -> p h o", o=1)
                                 .broadcast_to([128, HQ + 1, HD]))
                        nc.vector.tensor_tensor(out=hview(qs[:]),
                                                in0=hview(qk),
                                                in1=inv_b, op=mul)
                        nc.vector.tensor_tensor(
                            out=hview(m1[:]), in0=hview(qs[:]),
                            in1=hview(cs_c[:, j, :]), op=mul)
                        nc.gpsimd.tensor_tensor(
                            out=hview(m2[:])[:, :, 0:32],
                            in0=hview(qs[:])[:, :, 32:HD],
                            in1=hview(cs_s[:, j, :])[:, :, 0:32], op=mul)
                        nc.gpsimd.tensor_tensor(
                            out=hview(m2[:])[:, :, 32:HD],
                            in0=hview(qs[:])[:, :, 0:32],
                            in1=hview(cs_s[:, j, :])[:, :, 32:HD], op=mul)
                        nc.vector.tensor_tensor(out=roped[:], in0=m1[:],
                                                in1=m2[:], op=add)
                        if debug:
                            nc.sync.dma_start(dbg["d_roped"][:, j, :],
                                              roped[:].bitcast(F32))
                        pend_tr.append((j, roped))

                    # software pipeline: sqrt lags stats by one tile, rope
                    # by two, transposes by three, so no engine queue blocks
                    # on work another engine hasn't finished yet
                    for j in range(NT):
                        do_stats(j)
                        if j >= 1:
                            do_sqrt(j - 1)
                        if j >= 2:
                            do_rope(j - 2)
                        if j >= 3:
                            do_transposes(*pend_tr.pop(0))
                    do_sqrt(NT - 1)
                    do_rope(NT - 2)
                    do_transposes(*pend_tr.pop(0))
                    do_rope(NT - 1)
                    while pend_tr:
                        tail = do_transposes(*pend_tr.pop(0))

                def fin():
                    nonlocal prev_tail
                    if serial and prev_tail is not None:
                        for e in entries:
                            add_dep_helper(e.ins, prev_tail.ins, sync=True,
                                           reason="serialize reps")
                    prev_tail = tail
                if debug:
                    nc.sync.dma_start(dbg["d_qT"][:], qT_all[:].bitcast(F32))
                    nc.sync.dma_start(dbg["d_kT"][:], kT_all[:].bitcast(F32))

                if "B" not in phases:
                    fin()
                    continue

                # ---- phase B: attention, head pairs in PE row groups ----
                a2a_in = [dram.tile([N_CORES, 2 * HD, TS], BF16,
                                    name=f"a2i{p}_{_rep}", tag=f"a2i{p}_{_rep}")
                          for p in range(2)]
                a2a_out = [dram.tile([N_CORES, 2 * HD, TS], BF16,
                                     name=f"a2o{p}_{_rep}", tag=f"a2o{p}_{_rep}")
                           for p in range(2)]
                colls = []
                with ExitStack() as pb:
                    epool = pb.enter_context(tc.tile_pool(name="expp", bufs=4))
                    bscr = pb.enter_context(tc.tile_pool(name="scrB", bufs=2))
                    ps_st = pb.enter_context(
                        tc.tile_pool(name="ps_st", bufs=4, space="PSUM"))
                    ps_ot = pb.enter_context(
                        tc.tile_pool(name="ps_ot", bufs=2, space="PSUM"))

                    for p in range(2):
                        pots = [ps_ot.tile([128, T], F32, name=f"pot{p}{hh}",
                                           tag="pot") for hh in range(2)]
                        ots = [bscr.tile([128, T], BF16, name=f"ot{hh}",
                                         tag="ot") for hh in range(2)]

                        def do_tail(a, b, hop_eng):
                            # normalize + stage O^T for psum cols [a, b):
                            # rcp of den (psum-direct at base 0) -> partition
                            # hop of the small rcp tile -> multiply. The hop
                            # engine is sync for the mid-pair chunk (a scalar
                            # -queue DMA would block later exps on the Act
                            # SEQ) and scalar for the end-of-pair chunk.
                            wdt = b - a
                            for hh in range(2):
                                rcp = bscr.tile([HD, 512], F32,
                                                name="rcp", tag="rcp")
                                nc.vector.reciprocal_approx_fast(
                                    out=rcp[:, 0:wdt], in_=pots[hh][0:HD, a:b])
                                rcp_hi = bscr.tile([128, 512], F32,
                                                   name="rcphi", tag="rcphi")
                                hop_eng.dma_start(rcp_hi[HD:128, 0:wdt],
                                                  rcp[:, 0:wdt])
                                nc.vector.tensor_tensor(
                                    out=ots[hh][HD:128, a:b],
                                    in0=pots[hh][HD:128, a:b],
                                    in1=rcp_hi[HD:128, 0:wdt], op=mul)
                                j0, j1 = a // TS, b // TS
                                nc.sync.dma_start(
                                    a2a_in[p][j0:j1, HD * hh:HD * (hh + 1), :]
                                    .rearrange("j d t -> d j t"),
                                    ots[hh][HD:128, a:b].rearrange(
                                        "d (j t) -> d j t", j=j1 - j0))

                        def do_av(i, ets):
                            t0 = 128 * i
                            for hh in range(2):
                                for (a, b) in ((0, 512), (512, 1024)):
                                    if t0 >= b:
                                        continue
                                    lo = max(a, t0)
                                    nc.tensor.matmul(
                                        pots[hh][:, lo:b],
                                        qv[i][:, 0:128],
                                        ets[hh][:, lo - t0:b - t0],
                                        start=(i == 0),
                                        stop=(i == min(b // 128, NT) - 1))

                        # AV matmuls lag the score matmuls by one s-tile so
                        # the PE never queues behind an exp it must wait for
                        pend_av = []
                        for i in range(NT):
                            t0 = 128 * i
                            w = T - t0
                            ets = [epool.tile([128, T], BF16, name=f"et{hh}",
                                              tag="et") for hh in range(2)]
                            # paired score matmuls in different PE row
                            # groups, 512-col psum chunks (1 bank each) so
                            # the pipeline holds 4 outstanding chunks
                            for (c0, cw) in chunks_for(w):
                                psts = [ps_st.tile([128, 512], F32,
                                                   name="pst", tag="pst")
                                        for hh in range(2)]
                                for hh in range(2):
                                    p0 = 64 * hh
                                    nc.tensor.matmul(
                                        psts[hh][:, 0:cw],
                                        kT_all[p0:p0 + HD, t0:t0 + 128],
                                        qT_all[p0:p0 + HD, p,
                                               t0 + c0:t0 + c0 + cw],
                                        start=True, stop=True)
                                for hh in range(2):
                                    nc.scalar.activation(
                                        ets[hh][:, c0:c0 + cw],
                                        psts[hh][:, 0:cw],
                                        mybir.ActivationFunctionType.Exp,
                                        scale=float(1.0 / np.sqrt(HD)))
                                    if c0 == 0:
                                        # causal mask on the diagonal block
                                        meng = nc.gpsimd if p == 0 else nc.vector
                                        meng.tensor_tensor(
                                            out=ets[hh][:, 0:128],
                                            in0=ets[hh][:, 0:128],
                                            in1=mask_sb[:, i, :], op=mul)
                            pend_av.append((i, ets))
                            if len(pend_av) > 1:
                                do_av(*pend_av.pop(0))
                                if i == 4:
                                    # psum cols 0-511 complete after AV(3):
                                    # run half the normalize tail mid-pair
                                    do_tail(0, 512, nc.sync)
                        do_av(*pend_av.pop(0))
                        do_tail(512, 1024, nc.scalar)
                        if debug and p == 0:
                            dpot = bscr.tile([128, T], F32, name="dpot", tag="dpot")
                            nc.vector.tensor_copy(dpot[:], pots[0][:])
                            nc.sync.dma_start(dbg["d_pot0"][:], dpot[:])
                            nc.sync.dma_start(dbg["d_ot0"][:],
                                              ots[0][HD:128, :].bitcast(F32))
                        if no_collective:
                            colls.append(nc.sync.dma_start(a2a_out[p][:],
                                                           a2a_in[p][:]))
                        else:
                            colls.append(nc.gpsimd.collective_compute(
                                "AllToAll", mybir.AluOpType.bypass,
                                replica_groups=[list(range(N_CORES))],
                                ins=[a2a_in[p].opt()], outs=[a2a_out[p].opt()]))
                tail = colls[-1]

                # ---- phase C: token-parallel out-projection ----
                # contraction row 256*src + 64*h + d: piece 0 (heads 0-1)
                # supplies even k-tiles, piece 1 (heads 2-3) odd k-tiles.
                if "C" not in phases:
                    fin()
                    continue
                with ExitStack() as pc:
                    agp = pc.enter_context(tc.tile_pool(name="agp", bufs=1))
                    oscr = pc.enter_context(tc.tile_pool(name="oscr", bufs=1))
                    ps_o = pc.enter_context(
                        tc.tile_pool(name="ps_o", bufs=1, space="PSUM"))
                    ps_w = pc.enter_context(
                        tc.tile_pool(name="ps_w", bufs=1, space="PSUM"))

                    def keep_warm(n, idx, fp32):
                        # dep-free matmuls bridge the PE-idle gap of an
                        # in-flight collective so the next real matmuls run
                        # at full clock (HAM stays engaged); sunk to DRAM so
                        # the chain can't be pruned
                        pw = ps_w.tile([128, 512], F32, name=f"pw{idx}",
                                       tag="pw")
                        ww, wx = (warm_wf, warm_xf) if fp32 else (warm_w, warm_x)
                        for k in range(n):
                            nc.tensor.matmul(pw[:], ww[:], wx[:],
                                             start=(k == 0), stop=(k == n - 1))
                        wsb = oscr.tile([1, 2], F32, name=f"wsb{idx}",
                                        tag=f"wsb{idx}")
                        nc.scalar.copy(wsb[:], pw[0:1, 0:2])
                        nc.sync.dma_start(warm_sink[:], wsb[:])

                    pos = [ps_o.tile([128, 512], F32, name=f"po{c}",
                                     tag=f"po{c}") for c in range(4)]
                    for p in range(2):
                        keep_warm(*((12, 0, False) if p == 0 else (16, 1, True)))
                        agt = agp.tile([128, NT, TS], BF16, name=f"agt{p}",
                                       tag=f"agt{p}")
                        nc.scalar.dma_start(
                            agt[:], a2a_out[p].rearrange("s p t -> p s t"))
                        for s in range(NT):
                            for c in range(4):
                                nc.tensor.matmul(
                                    pos[c],
                                    agt[:, s, :],
                                    wo_sb[:, 2 * s + p, 512 * c:512 * (c + 1)],
                                    start=(p == 0 and s == 0),
                                    stop=(p == 1 and s == NT - 1))
                    out_sb = oscr.tile([128, 4, 512], F32, name="out_sb",
                                       tag="out_sb")
                    for c in range(4):
                        if c % 2 == 0:
                            nc.scalar.copy(out_sb[:, c, :], pos[c][:])
                        else:
                            nc.vector.tensor_copy(out_sb[:, c, :], pos[c][:])
                        tail = nc.sync.dma_start(
                            outT[:, 512 * c:512 * (c + 1)], out_sb[:, c, :])
                fin()

    nc.compile()
    _NC_CACHE[key] = nc
    return nc


def prep_in_maps(inputs):
    """Host-side sharding + layout prep. Returns per-core input maps."""
    x = np.asarray(inputs["x"], dtype=np.float32)
    mask = np.asarray(inputs["mask"])
    cos = np.asarray(inputs["cos"], dtype=np.float32)
    sin = np.asarray(inputs["sin"], dtype=np.float32)
    Wq = np.asarray(inputs["Wq"], dtype=np.float32)
    Wk = np.asarray(inputs["Wk"], dtype=np.float32)
    Wv = np.asarray(inputs["Wv"], dtype=np.float32)
    Wo = np.asarray(inputs["Wo"], dtype=np.float32)
    qw = np.asarray(inputs["q_norm_w"], dtype=np.float32)
    kw = np.asarray(inputs["k_norm_w"], dtype=np.float32)

    bf = ml_dtypes.bfloat16
    xT = np.ascontiguousarray(x.T).astype(bf)

    # norm weights folded into per-(q/k) rope tables; sin table carries the
    # rotate-half signs: out[i] = w[i]*(q[i]*cos[i] + rot[i]*sin[i]),
    # rot[i] = -q[i+32] (i<32) else q[i-32]
    sgn = np.concatenate([-np.ones(HD // 2, np.float32),
                          np.ones(HD // 2, np.float32)])
    cos_q = cos * qw[None, :]
    sin_q = sin * (sgn * qw)[None, :]
    cos_k = cos * kw[None, :]
    sin_k = sin * (sgn * kw)[None, :]
    cos_all = np.ascontiguousarray(
        np.concatenate([cos_q] * HQ + [cos_k], axis=1)).astype(bf)
    sin_all = np.ascontiguousarray(
        np.concatenate([sin_q] * HQ + [sin_k], axis=1)).astype(bf)

    # multiplicative mask for diagonal blocks, in S^T layout:
    # maskmul[s', i, t'] = 0 where mask[128i+t', 128i+s'] else 1
    mm = np.ones((128, NT, 128), np.float32)
    for i in range(NT):
        blk = mask[128 * i:128 * (i + 1), 128 * i:128 * (i + 1)]
        mm[:, i, :] = np.where(blk.T, 0.0, 1.0).astype(np.float32)
    maskmul = mm.astype(bf)

    # token-parallel out-projection: full Wo^T on every core; its
    # contraction row order 256*src + 64*h + d matches the AllToAll output
    woT = np.ascontiguousarray(Wo.T).astype(bf)                # [2048, 2048]
    in_maps = []
    for r in range(N_CORES):
        wq_r = Wq[E * r:E * (r + 1), :]          # [256, 2048]
        wk_r = Wk[HD * r:HD * (r + 1), :]        # [64, 2048]
        wv_r = Wv[HD * r:HD * (r + 1), :]        # [64, 2048]
        # V first: the projection psum is [V | Q | K], evicted to qv cols
        # 64:448 behind the ones block
        wqkv = np.ascontiguousarray(
            np.concatenate([wv_r, wq_r, wk_r], axis=0).T).astype(bf)
        in_maps.append({
            "xT": xT, "wqkv": wqkv, "wo": woT,
            "cos_all": cos_all, "sin_all": sin_all,
            "maskmul": maskmul,
        })
    return in_maps


def kernel(**inputs) -> np.ndarray:
    nc = build_nc()
    in_maps = prep_in_maps(inputs)
    res = run_bass_kernel_spmd(nc, in_maps, list(range(N_CORES)))
    out = np.empty((T, D), dtype=np.float32)
    for r in range(N_CORES):
        out[TS * r:TS * (r + 1), :] = res.results[r]["outT"]
    return out


# revision 65
# speedup vs baseline: 1.0068x; 1.0065x over previous
"""Trainium2 Bass kernel for GroupQueryAttention (T=1024, D=2048, H=32, KV=8, HD=64).

Sharding: tensor-parallel over heads across 8 NeuronCores. Core r owns q-heads
4r..4r+3 and kv-head r (split Wq/Wk/Wv output dims). x is replicated. The
per-core attention outputs are exchanged with an AllToAll (split in two
pieces, overlapped with compute) and the out-projection is token-parallel:
each core computes its 128-token slice of the final [1024, 2048] output with
the full 2048-deep contraction.

Per-core dataflow:
  A) fused QKV projection run k-outer over 8 concurrent psum groups so the
     PE paces with the x/w DMA stream (inputs split per 2-k-tile chunk over
     both HWDGE queues); then a software-pipelined RMSNorm + RoPE chain in
     bf16 on DVE/GpSimd (norm weight folded into host-side cos/sin tables,
     stats -> sqrt -> rope -> PE-transpose each lagged one t-tile).
  B) attention in transposed layout, heads processed in pairs (h even at
     partitions 0-63, h odd at 64-127) so the K=64 score matmuls of the two
     heads run concurrently in different PE row groups; AV matmuls lag the
     scores by one s-tile so the PE never queues behind an exp. The V tile
     carries 64 ones-columns, so the AV matmul emits O^T (psum rows 0-63)
     and the softmax denominator replicated 64x (rows 64-127) in one
     stream; the normalize tail (den copy, partition-hop DMA, approx
     reciprocal, multiply) runs per 512-col chunk -- the first half
     mid-pair, off the critical path.
  C) two-piece AllToAll (heads 0-1 staged right after pair 1 and overlapped
     with pair-2 compute; heads 2-3 after). The out-projection accumulates
     piece-1 k-tiles (even) while piece 2 is on the wire. Dep-free
     warm-keeper matmuls on scratch data bridge the PE-idle collective
     gaps so the out-projection runs at full clock (HAM stays engaged).
"""
import sys
import numpy as np
import ml_dtypes

sys.path.insert(0, "/opt/trn_rl_repo")

import concourse.bass as bass
import concourse.mybir as mybir
import concourse.tile as tile
import concourse.bacc as bacc
from concourse.bass_utils import run_bass_kernel_spmd
from concourse.masks import make_identity
from concourse.tile_rust import add_dep_helper
from contextlib import ExitStack

N_CORES = 8
T, D, H, KVH, HD = 1024, 2048, 32, 8, 64
GROUP = H // KVH          # 4 q-heads per kv-head
HQ = H // N_CORES         # 4 q-heads per core
E = HQ * HD               # 256 = per-core q width
QKV = E + HD + HD         # 384 = fused projection width
NT = T // 128             # 8 t-tiles
NK = D // 128             # 16 contraction tiles
EPS = 1e-6
F32 = mybir.dt.float32
BF16 = mybir.dt.bfloat16
TS = T // N_CORES         # 128 = per-core token slice of the output

_NC_CACHE = {}


def chunks_for(width):
    out = []
    for a in range(0, width, 512):
        out.append((a, min(width - a, 512)))
    return out


def build_nc(debug=False, reps=1, no_collective=False, phases="ABC",
             skip_in_dma=False, serial=False):
    key = ("nc", debug, reps, no_collective, phases, skip_in_dma, serial)
    if key in _NC_CACHE:
        return _NC_CACHE[key]
    nc = bacc.Bacc("TRN2", target_bir_lowering=False, debug=False,
                   num_devices=N_CORES)
    mul = mybir.AluOpType.mult
    add = mybir.AluOpType.add

    xT = nc.dram_tensor("xT", [D, T], BF16, kind="ExternalInput").ap()
    wqkv = nc.dram_tensor("wqkv", [D, QKV], BF16, kind="ExternalInput").ap()
    wo = nc.dram_tensor("wo", [D, D], BF16, kind="ExternalInput").ap()
    cos_all = nc.dram_tensor("cos_all", [T, E + HD], BF16,
                             kind="ExternalInput").ap()
    sin_all = nc.dram_tensor("sin_all", [T, E + HD], BF16,
                             kind="ExternalInput").ap()
    maskmul = nc.dram_tensor("maskmul", [128, NT, 128], BF16,
                             kind="ExternalInput").ap()
    outT = nc.dram_tensor("outT", [TS, D], F32, kind="ExternalOutput").ap()
    # sink for the PE warm-keeper chains (never read by the host)
    warm_sink = nc.dram_tensor("warm_sink", [1, 2], F32,
                               kind="ExternalOutput").ap()
    dbg = {}
    if debug:
        for nm, shape in [("d_qv", [128, NT, 448]), ("d_roped", [128, NT, E + HD]),
                          ("d_qT", [128, 2, T]), ("d_kT", [128, T]),
                          ("d_exp00", [128, T]), ("d_exp03", [128, T]),
                          ("d_pot0", [128, T]), ("d_rcp0", [64, T]),
                          ("d_ot0", [64, T])]:
            dbg[nm] = nc.dram_tensor(nm, shape, F32, kind="ExternalOutput").ap()

    hview = lambda ap: ap.rearrange("p (h d) -> p h d", h=HQ + 1)

    with tile.TileContext(nc) as tc:
        with ExitStack() as top:
            persist = top.enter_context(tc.tile_pool(name="persist", bufs=1))
            dram = top.enter_context(tc.tile_pool(name="dram", bufs=1, space="DRAM"))

            # ---- one-time constants ----
            ident = persist.tile([128, 128], BF16)
            make_identity(nc, ident[:])
            eps_t = persist.tile([128, 1], F32)
            nc.gpsimd.memset(eps_t[:], EPS)
            # scratch for the PE warm-keeper chains (values irrelevant)
            warm_w = persist.tile([128, 128], BF16)
            nc.gpsimd.memset(warm_w[:], 0.0)
            warm_x = persist.tile([128, 512], BF16)
            nc.gpsimd.memset(warm_x[:], 0.0)
            warm_wf = persist.tile([128, 128], F32)
            nc.gpsimd.memset(warm_wf[:], 0.0)
            warm_xf = persist.tile([128, 512], F32)
            nc.gpsimd.memset(warm_xf[:], 0.0)

            # ---- persistent (per-rep-rewritten) tiles ----
            qT_all = persist.tile([128, 2, T], BF16, name="qT_all")
            kT_all = persist.tile([128, T], BF16, name="kT_all")
            # per-t-tile fused [ones 0:64 | v 64:128 | qk 128:448]; the
            # AV lhsT is cols 0:128, so the softmax denominator comes out
            # at psum partitions 0-63 (base 0 -- custom-DVE-op legal) and
            # O^T at 64-127
            qv = [persist.tile([128, 448], BF16, name=f"qv{j}")
                  for j in range(NT)]
            cs_c = persist.tile([128, NT, E + HD], BF16, name="cs_c")
            cs_s = persist.tile([128, NT, E + HD], BF16, name="cs_s")
            mask_sb = persist.tile([128, NT, 128], BF16, name="mask_sb")
            wo_sb = persist.tile([128, NK, D], BF16, name="wo_sb")
            xt_g = [persist.tile([128, 2, T], BF16, name=f"xt{g}")
                    for g in range(NK // 2)]
            wq_g = [persist.tile([128, 2, QKV], BF16, name=f"wq{g}")
                    for g in range(NK // 2)]

            xT_v = xT.rearrange("(k p) t -> p k t", p=128)
            wq_v = wqkv.rearrange("(k p) n -> p k n", p=128)
            wo_v = wo.rearrange("(k p) n -> p k n", p=128)

            cos_v = cos_all.rearrange("(j p) n -> p j n", p=128)
            sin_v = sin_all.rearrange("(j p) n -> p j n", p=128)

            def load_inputs():
                ins = []
                # rope tables for the first two t-tiles up front (tiny);
                # the rest after x/w so the projection isn't DMA-starved
                ins.append(nc.scalar.dma_start(cs_c[:, 0:2, :],
                                               cos_v[:, 0:2, :]))
                ins.append(nc.scalar.dma_start(cs_s[:, 0:2, :],
                                               sin_v[:, 0:2, :]))
                # x^T and W_qkv^T split per 2-k-tile group, alternating
                # the two HWDGE queues; x^T additionally split into t-halves
                # -- t-tiles 0-3 of the projection need only cols 0:512 of
                # every contraction chunk, so their psum groups (and the
                # norm/rope chains behind them) finish ~1.3MB of DMA earlier
                for g in range(NK // 2):
                    eng = nc.sync if g % 2 == 0 else nc.scalar
                    ins.append(eng.dma_start(xt_g[g][:, :, 0:512],
                                             xT_v[:, 2 * g:2 * g + 2, 0:512]))
                    ins.append(eng.dma_start(wq_g[g][:], wq_v[:, 2 * g:2 * g + 2, :]))
                ins.append(nc.scalar.dma_start(cs_c[:, 2:4, :],
                                               cos_v[:, 2:4, :]))
                ins.append(nc.scalar.dma_start(cs_s[:, 2:4, :],
                                               sin_v[:, 2:4, :]))
                for g in range(NK // 2):
                    eng = nc.sync if g % 2 == 0 else nc.scalar
                    ins.append(eng.dma_start(xt_g[g][:, :, 512:T],
                                             xT_v[:, 2 * g:2 * g + 2, 512:T]))
                ins.append(nc.scalar.dma_start(cs_c[:, 4:NT, :],
                                               cos_v[:, 4:NT, :]))
                ins.append(nc.scalar.dma_start(cs_s[:, 4:NT, :],
                                               sin_v[:, 4:NT, :]))
                ins.append(nc.scalar.dma_start(mask_sb[:], maskmul[:]))
                # 8MB Wo load in the background, after x/w on both queues
                for q in range(8):
                    eng = nc.sync if q % 2 == 0 else nc.scalar
                    ins.append(eng.dma_start(
                        wo_sb[:, 2 * q:2 * (q + 1), :],
                        wo_v[:, 2 * q:2 * (q + 1), :]))
                return ins

            prev_tail = None
            for _rep in range(reps):
                entries = load_inputs()
                # ones columns for the AV denominator rows
                for j in range(NT):
                    nc.gpsimd.memset(qv[j][:, 0:64], 1.0)

                # ---- phase A: QKV projection + norm + rope + transposes ----
                pend_tr = []  # transposes lagged one t-tile to keep PE dense

                def do_transposes(j, roped):
                    for m in range(2):
                        ptr = ps_tr.tile([128, 128], BF16, name="ptr", tag="ptr")
                        nc.tensor.transpose(
                            ptr[:], roped[:, 128 * m:128 * (m + 1)], ident[:])
                        nc.scalar.copy(qT_all[:, m, 128 * j:128 * (j + 1)], ptr[:])
                    ptrk = ps_tr.tile([64, 128], BF16, name="ptrk", tag="ptrk")
                    nc.tensor.transpose(ptrk[:], roped[:, E:E + HD], ident[:])
                    nc.scalar.copy(kT_all[0:HD, 128 * j:128 * (j + 1)], ptrk[:])
                    # per-tile dup into partitions 64..127 for odd-head MMs
                    return nc.sync.dma_start(
                        kT_all[HD:128, 128 * j:128 * (j + 1)],
                        kT_all[0:HD, 128 * j:128 * (j + 1)])

                # A-1: QKV projection, k-outer over 8 concurrent psum
                # groups -- each matmul needs only contraction chunk k, so
                # the PE paces with the x/w DMA stream instead of
                # head-of-line blocking on the last chunk.
                with ExitStack() as pa0:
                    ps_qkv = pa0.enter_context(
                        tc.tile_pool(name="ps_qkv", bufs=1, space="PSUM"))
                    pqs = [ps_qkv.tile([128, QKV], F32, name=f"pq{j}",
                                       tag=f"pq{j}") for j in range(NT)]
                    # two passes: t-tiles 0-3 (x cols 0:512) fully
                    # accumulate and evict while the second x half streams
                    for (j0, j1) in ((0, NT // 2), (NT // 2, NT)):
                        for k in range(NK):
                            for j in range(j0, j1):
                                nc.tensor.matmul(
                                    pqs[j][:],
                                    xt_g[k // 2][:, k % 2,
                                                 128 * j:128 * (j + 1)],
                                    wq_g[k // 2][:, k % 2, :],
                                    start=(k == 0), stop=(k == NK - 1))
                        for j in range(j0, j1):
                            # single psum eviction: Q|K for rope + V slice
                            nc.scalar.copy(qv[j][:, 64:448], pqs[j][:])
                            if debug:
                                nc.sync.dma_start(dbg["d_qv"][:, j, :],
                                                  qv[j][:].bitcast(F32))

                # A-2: norm + rope + transposes (psum freed by A-1)
                with ExitStack() as pa:
                    scrA = pa.enter_context(tc.tile_pool(name="scrA", bufs=4))
                    ps_tr = pa.enter_context(
                        tc.tile_pool(name="ps_tr", bufs=2, space="PSUM"))

                    stats = {}

                    def do_stats(j):
                        # sum of squares per head for t-tile j (DVE)
                        qk = qv[j][:, 128:448]
                        sq = scrA.tile([128, E + HD], BF16, name="sq", tag="sq")
                        nc.vector.tensor_tensor(out=sq[:], in0=qk, in1=qk, op=mul)
                        ssq = scrA.tile([128, 8], F32, name="ssq", tag="ssq")
                        nc.vector.tensor_reduce(
                            out=ssq[:, 0:HQ + 1],
                            in_=sq[:].rearrange("p (h d) -> p h d", h=HQ + 1),
                            axis=mybir.AxisListType.X, op=add)
                        stats[j] = ssq

                    rmss = {}

                    def do_sqrt(j):
                        ssq = stats.pop(j)
                        rms = scrA.tile([128, 8], F32, name="rms", tag="rms")
                        nc.scalar.activation(rms[:, 0:HQ + 1], ssq[:, 0:HQ + 1],
                                             mybir.ActivationFunctionType.Sqrt,
                                             scale=1.0 / HD, bias=eps_t[:])
                        rmss[j] = rms

                    def do_rope(j):
                        # inv scale then rope (DVE/Pool)
                        qk = qv[j][:, 128:448]
                        rms = rmss.pop(j)
                        inv = scrA.tile([128, 8], F32, name="inv", tag="inv")
                        nc.vector.reciprocal(inv[:, 0:HQ + 1], rms[:, 0:HQ + 1])
                        # rope: qs = qk * inv[head]; m1 = qs*cos (gpsimd);
                        # m2 = swap(qs)*sin (gpsimd); roped = m1 + m2
                        qs = scrA.tile([128, E + HD], BF16, name="qs", tag="qs")
                        m1 = scrA.tile([128, E + HD], BF16, name="m1", tag="m1")
                        m2 = scrA.tile([128, E + HD], BF16, name="m2", tag="m2")
                        roped = scrA.tile([128, E + HD], BF16, name="roped",
                                          tag="roped")
                        inv_b = (inv[:, 0:HQ + 1]
                                 .rearrange("p (h o) -> p h o", o=1)
                                 .broadcast_to([128, HQ + 1, HD]))
                        nc.vector.tensor_tensor(out=hview(qs[:]),
                                                in0=hview(qk),
                                                in1=inv_b, op=mul)
                        nc.vector.tensor_tensor(
                            out=hview(m1[:]), in0=hview(qs[:]),
                            in1=hview(cs_c[:, j, :]), op=mul)
                        nc.gpsimd.tensor_tensor(
                            out=hview(m2[:])[:, :, 0:32],
                            in0=hview(qs[:])[:, :, 32:HD],
                            in1=hview(cs_s[:, j, :])[:, :, 0:32], op=mul)
                        nc.gpsimd.tensor_tensor(
                            out=hview(m2[:])[:, :, 32:HD],
                            in0=hview(qs[:])[:, :, 0:32],
                            in1=hview(cs_s[:, j, :])[:, :, 32:HD], op=mul)
                        nc.vector.tensor_tensor(out=roped[:], in0=m1[:],
                                                in1=m2[:], op=add)
                        if debug:
                            nc.sync.dma_start(dbg["d_roped"][:, j, :],
                                              roped[:].bitcast(F32))
                        pend_tr.append((j, roped))

                    # software pipeline: sqrt lags stats by one tile, rope
                    # by two, transposes by three, so no engine queue blocks
                    # on work another engine hasn't finished yet
                    for j in range(NT):
                        do_stats(j)
                        if j >= 1:
                            do_sqrt(j - 1)
                        if j >= 2:
                            do_rope(j - 2)
                        if j >= 3:
                            do_transposes(*pend_tr.pop(0))
                    do_sqrt(NT - 1)
                    do_rope(NT - 2)
                    do_transposes(*pend_tr.pop(0))
                    do_rope(NT - 1)
                    while pend_tr:
                        tail = do_transposes(*pend_tr.pop(0))

                def fin():
                    nonlocal prev_tail
                    if serial and prev_tail is not None:
                        for e in entries:
                            add_dep_helper(e.ins, prev_tail.ins, sync=True,
                                           reason="serialize reps")
                    prev_tail = tail
                if debug:
                    nc.sync.dma_start(dbg["d_qT"][:], qT_all[:].bitcast(F32))
                    nc.sync.dma_start(dbg["d_kT"][:], kT_all[:].bitcast(F32))

                if "B" not in phases:
                    fin()
                    continue

                # ---- phase B: attention, head pairs in PE row groups ----
                a2a_in = [dram.tile([N_CORES, 2 * HD, TS], BF16,
                                    name=f"a2i{p}_{_rep}", tag=f"a2i{p}_{_rep}")
                          for p in range(2)]
                a2a_out = [dram.tile([N_CORES, 2 * HD, TS], BF16,
                                     name=f"a2o{p}_{_rep}", tag=f"a2o{p}_{_rep}")
                           for p in range(2)]
                colls = []
                with ExitStack() as pb:
                    epool = pb.enter_context(tc.tile_pool(name="expp", bufs=4))
                    bscr = pb.enter_context(tc.tile_pool(name="scrB", bufs=2))
                    ps_st = pb.enter_context(
                        tc.tile_pool(name="ps_st", bufs=4, space="PSUM"))
                    ps_ot = pb.enter_context(
                        tc.tile_pool(name="ps_ot", bufs=2, space="PSUM"))

                    for p in range(2):
                        pots = [ps_ot.tile([128, T], F32, name=f"pot{p}{hh}",
                                           tag="pot") for hh in range(2)]
                        ots = [bscr.tile([128, T], BF16, name=f"ot{hh}",
                                         tag="ot") for hh in range(2)]

                        def do_tail(a, b, hop_eng):
                            # normalize + stage O^T for psum cols [a, b):
                            # rcp of den (psum-direct at base 0) -> partition
                            # hop of the small rcp tile -> multiply. The hop
                            # engine is sync for the mid-pair chunk (a scalar
                            # -queue DMA would block later exps on the Act
                            # SEQ) and scalar for the end-of-pair chunk.
                            wdt = b - a
                            for hh in range(2):
                                rcp = bscr.tile([HD, 512], F32,
                                                name="rcp", tag="rcp")
                                nc.vector.reciprocal_approx_fast(
                                    out=rcp[:, 0:wdt], in_=pots[hh][0:HD, a:b])
                                rcp_hi = bscr.tile([128, 512], F32,
                                                   name="rcphi", tag="rcphi")
                                hop_eng.dma_start(rcp_hi[HD:128, 0:wdt],
                                                  rcp[:, 0:wdt])
                                nc.vector.tensor_tensor(
                                    out=ots[hh][HD:128, a:b],
                                    in0=pots[hh][HD:128, a:b],
                                    in1=rcp_hi[HD:128, 0:wdt], op=mul)
                                j0, j1 = a // TS, b // TS
                                nc.sync.dma_start(
                                    a2a_in[p][j0:j1, HD * hh:HD * (hh + 1), :]
                                    .rearrange("j d t -> d j t"),
                                    ots[hh][HD:128, a:b].rearrange(
                                        "d (j t) -> d j t", j=j1 - j0))

                        def do_av(i, ets):
                            t0 = 128 * i
                            for hh in range(2):
                                for (a, b) in ((0, 512), (512, 1024)):
                                    if t0 >= b:
                                        continue
                                    lo = max(a, t0)
                                    nc.tensor.matmul(
                                        pots[hh][:, lo:b],
                                        qv[i][:, 0:128],
                                        ets[hh][:, lo - t0:b - t0],
                                        start=(i == 0),
                                        stop=(i == min(b // 128, NT) - 1))

                        # AV matmuls lag the score matmuls by one s-tile so
                        # the PE never queues behind an exp it must wait for
                        pend_av = []
                        for i in range(NT):
                            t0 = 128 * i
                            w = T - t0
                            ets = [epool.tile([128, T], BF16, name=f"et{hh}",
                                              tag="et") for hh in range(2)]
                            # paired score matmuls in different PE row
                            # groups, 512-col psum chunks (1 bank each) so
                            # the pipeline holds 4 outstanding chunks
                            for (c0, cw) in chunks_for(w):
                                psts = [ps_st.tile([128, 512], F32,
                                                   name="pst", tag="pst")
                                        for hh in range(2)]
                                for hh in range(2):
                                    p0 = 64 * hh
                                    nc.tensor.matmul(
                                        psts[hh][:, 0:cw],
                                        kT_all[p0:p0 + HD, t0:t0 + 128],
                                        qT_all[p0:p0 + HD, p,
                                               t0 + c0:t0 + c0 + cw],
                                        start=True, stop=True)
                                for hh in range(2):
                                    nc.scalar.activation(
                                        ets[hh][:, c0:c0 + cw],
                                        psts[hh][:, 0:cw],
                                        mybir.ActivationFunctionType.Exp,
                                        scale=float(1.0 / np.sqrt(HD)))
                                    if c0 == 0:
                                        # causal mask on the diagonal block
                                        meng = nc.gpsimd if p == 0 else nc.vector
                                        meng.tensor_tensor(
                                            out=ets[hh][:, 0:128],
                                            in0=ets[hh][:, 0:128],
                                            in1=mask_sb[:, i, :], op=mul)
                            pend_av.append((i, ets))
                            if len(pend_av) > 1:
                                do_av(*pend_av.pop(0))
                                if i == 4:
                                    # psum cols 0-511 complete after AV(3):
                                    # run half the normalize tail mid-pair
                                    do_tail(0, 512, nc.sync)
                        do_av(*pend_av.pop(0))
                        do_tail(512, 1024, nc.scalar)
                        if debug and p == 0:
                            dpot = bscr.tile([128, T], F32, name="dpot", tag="dpot")
                            nc.vector.tensor_copy(dpot[:], pots[0][:])
                            nc.sync.dma_start(dbg["d_pot0"][:], dpot[:])
                            nc.sync.dma_start(dbg["d_ot0"][:],
                                              ots[0][HD:128, :].bitcast(F32))
                        if no_collective:
                            colls.append(nc.sync.dma_start(a2a_out[p][:],
                                                           a2a_in[p][:]))
                        else:
                            colls.append(nc.gpsimd.collective_compute(
                                "AllToAll", mybir.AluOpType.bypass,
                                replica_groups=[list(range(N_CORES))],
                                ins=[a2a_in[p].opt()], outs=[a2a_out[p].opt()]))
                tail = colls[-1]

                # ---- phase C: token-parallel out-projection ----
                # contraction row 256*src + 64*h + d: piece 0 (heads 0-1)
                # supplies even k-tiles, piece 1 (heads 2-3) odd k-tiles.
                if "C" not in phases:
                    fin()
                    continue
                with ExitStack() as pc:
                    agp = pc.enter_context(tc.tile_pool(name="agp", bufs=1))
                    oscr = pc.enter_context(tc.tile_pool(name="oscr", bufs=1))
                    ps_o = pc.enter_context(
                        tc.tile_pool(name="ps_o", bufs=1, space="PSUM"))
                    ps_w = pc.enter_context(
                        tc.tile_pool(name="ps_w", bufs=1, space="PSUM"))

                    def keep_warm(n, idx, fp32):
                        # dep-free matmuls bridge the PE-idle gap of an
                        # in-flight collective so the next real matmuls run
                        # at full clock (HAM stays engaged); sunk to DRAM so
                        # the chain can't be pruned
                        pw = ps_w.tile([128, 512], F32, name=f"pw{idx}",
                                       tag="pw")
                        ww, wx = (warm_wf, warm_xf) if fp32 else (warm_w, warm_x)
                        for k in range(n):
                            nc.tensor.matmul(pw[:], ww[:], wx[:],
                                             start=(k == 0), stop=(k == n - 1))
                        wsb = oscr.tile([1, 2], F32, name=f"wsb{idx}",
                                        tag=f"wsb{idx}")
                        nc.scalar.copy(wsb[:], pw[0:1, 0:2])
                        nc.sync.dma_start(warm_sink[:], wsb[:])

                    pos = [ps_o.tile([128, 512], F32, name=f"po{c}",
                                     tag=f"po{c}") for c in range(4)]
                    for p in range(2):
                        keep_warm(*((12, 0, False) if p == 0 else (16, 1, True)))
                        agt = agp.tile([128, NT, TS], BF16, name=f"agt{p}",
                                       tag=f"agt{p}")
                        nc.scalar.dma_start(
                            agt[:], a2a_out[p].rearrange("s p t -> p s t"))
                        for s in range(NT):
                            for c in range(4):
                                nc.tensor.matmul(
                                    pos[c],
                                    agt[:, s, :],
                                    wo_sb[:, 2 * s + p, 512 * c:512 * (c + 1)],
                                    start=(p == 0 and s == 0),
                                    stop=(p == 1 and s == NT - 1))
                    out_sb = oscr.tile([128, 4, 512], F32, name="out_sb",
                                       tag="out_sb")
                    for c in range(4):
                        if c % 2 == 0:
                            nc.scalar.copy(out_sb[:, c, :], pos[c][:])
                        else:
                            nc.vector.tensor_copy(out_sb[:, c, :], pos[c][:])
                        tail = nc.sync.dma_start(
                            outT[:, 512 * c:512 * (c + 1)], out_sb[:, c, :])
                fin()

    nc.compile()
    _NC_CACHE[key] = nc
    return nc


def prep_in_maps(inputs):
    """Host-side sharding + layout prep. Returns per-core input maps."""
    x = np.asarray(inputs["x"], dtype=np.float32)
    mask = np.asarray(inputs["mask"])
    cos = np.asarray(inputs["cos"], dtype=np.float32)
    sin = np.asarray(inputs["sin"], dtype=np.float32)
    Wq = np.asarray(inputs["Wq"], dtype=np.float32)
    Wk = np.asarray(inputs["Wk"], dtype=np.float32)
    Wv = np.asarray(inputs["Wv"], dtype=np.float32)
    Wo = np.asarray(inputs["Wo"], dtype=np.float32)
    qw = np.asarray(inputs["q_norm_w"], dtype=np.float32)
    kw = np.asarray(inputs["k_norm_w"], dtype=np.float32)

    bf = ml_dtypes.bfloat16
    xT = np.ascontiguousarray(x.T).astype(bf)

    # norm weights folded into per-(q/k) rope tables; sin table carries the
    # rotate-half signs: out[i] = w[i]*(q[i]*cos[i] + rot[i]*sin[i]),
    # rot[i] = -q[i+32] (i<32) else q[i-32]
    sgn = np.concatenate([-np.ones(HD // 2, np.float32),
                          np.ones(HD // 2, np.float32)])
    cos_q = cos * qw[None, :]
    sin_q = sin * (sgn * qw)[None, :]
    cos_k = cos * kw[None, :]
    sin_k = sin * (sgn * kw)[None, :]
    cos_all = np.ascontiguousarray(
        np.concatenate([cos_q] * HQ + [cos_k], axis=1)).astype(bf)
    sin_all = np.ascontiguousarray(
        np.concatenate([sin_q] * HQ + [sin_k], axis=1)).astype(bf)

    # multiplicative mask for diagonal blocks, in S^T layout:
    # maskmul[s', i, t'] = 0 where mask[128i+t', 128i+s'] else 1
    mm = np.ones((128, NT, 128), np.float32)
    for i in range(NT):
        blk = mask[128 * i:128 * (i + 1), 128 * i:128 * (i + 1)]
        mm[:, i, :] = np.where(blk.T, 0.0, 1.0).astype(np.float32)
    maskmul = mm.astype(bf)

    # token-parallel out-projection: full Wo^T on every core; its
    # contraction row order 256*src + 64*h + d matches the AllToAll output
    woT = np.ascontiguousarray(Wo.T).astype(bf)                # [2048, 2048]
    in_maps = []
    for r in range(N_CORES):
        wq_r = Wq[E * r:E * (r + 1), :]          # [256, 2048]
        wk_r = Wk[HD * r:HD * (r + 1), :]        # [64, 2048]
        wv_r = Wv[HD * r:HD * (r + 1), :]        # [64, 2048]
        # V first: the projection psum is [V | Q | K], evicted to qv cols
        # 64:448 behind the ones block
        wqkv = np.ascontiguousarray(
            np.concatenate([wv_r, wq_r, wk_r], axis=0).T).astype(bf)
        in_maps.append({
            "xT": xT, "wqkv": wqkv, "wo": woT,
            "cos_all": cos_all, "sin_all": sin_all,
            "maskmul": maskmul,
        })
    return in_maps


def kernel(**inputs) -> np.ndarray:
    nc = build_nc()
    in_maps = prep_in_maps(inputs)
    res = run_bass_kernel_spmd(nc, in_maps, list(range(N_CORES)))
    out = np.empty((T, D), dtype=np.float32)
    for r in range(N_CORES):
        out[TS * r:TS * (r + 1), :] = res.results[r]["outT"]
    return out


# revision 68
# speedup vs baseline: 1.0122x; 1.0053x over previous
"""Trainium2 Bass kernel for GroupQueryAttention (T=1024, D=2048, H=32, KV=8, HD=64).

Sharding: tensor-parallel over heads across 8 NeuronCores. Core r owns q-heads
4r..4r+3 and kv-head r (split Wq/Wk/Wv output dims). x is replicated. The
per-core attention outputs are exchanged with an AllToAll (split in two
pieces, overlapped with compute) and the out-projection is token-parallel:
each core computes its 128-token slice of the final [1024, 2048] output with
the full 2048-deep contraction.

Per-core dataflow:
  A) fused QKV projection run k-outer over 8 concurrent psum groups so the
     PE paces with the x/w DMA stream (inputs split per 2-k-tile chunk over
     both HWDGE queues); then a software-pipelined RMSNorm + RoPE chain in
     bf16 on DVE/GpSimd (norm weight folded into host-side cos/sin tables,
     stats -> sqrt -> rope -> PE-transpose each lagged one t-tile).
  B) attention in transposed layout, heads processed in pairs (h even at
     partitions 0-63, h odd at 64-127) so the K=64 score matmuls of the two
     heads run concurrently in different PE row groups; AV matmuls lag the
     scores by one s-tile so the PE never queues behind an exp. The V tile
     carries 64 ones-columns, so the AV matmul emits O^T (psum rows 0-63)
     and the softmax denominator replicated 64x (rows 64-127) in one
     stream; the normalize tail (den copy, partition-hop DMA, approx
     reciprocal, multiply) runs per 512-col chunk -- the first half
     mid-pair, off the critical path.
  C) two-piece AllToAll (heads 0-1 staged right after pair 1 and overlapped
     with pair-2 compute; heads 2-3 after). The out-projection accumulates
     piece-1 k-tiles (even) while piece 2 is on the wire. Dep-free
     warm-keeper matmuls on scratch data bridge the PE-idle collective
     gaps so the out-projection runs at full clock (HAM stays engaged).
"""
import sys
import numpy as np
import ml_dtypes

sys.path.insert(0, "/opt/trn_rl_repo")

import concourse.bass as bass
import concourse.mybir as mybir
import concourse.tile as tile
import concourse.bacc as bacc
from concourse.bass_utils import run_bass_kernel_spmd
from concourse.masks import make_identity
from concourse.tile_rust import add_dep_helper
from contextlib import ExitStack

N_CORES = 8
T, D, H, KVH, HD = 1024, 2048, 32, 8, 64
GROUP = H // KVH          # 4 q-heads per kv-head
HQ = H // N_CORES         # 4 q-heads per core
E = HQ * HD               # 256 = per-core q width
QKV = E + HD + HD         # 384 = fused projection width
NT = T // 128             # 8 t-tiles
NK = D // 128             # 16 contraction tiles
EPS = 1e-6
F32 = mybir.dt.float32
BF16 = mybir.dt.bfloat16
TS = T // N_CORES         # 128 = per-core token slice of the output

_NC_CACHE = {}


def chunks_for(width):
    out = []
    for a in range(0, width, 512):
        out.append((a, min(width - a, 512)))
    return out


def build_nc(debug=False, reps=1, no_collective=False, phases="ABC",
             skip_in_dma=False, serial=False):
    key = ("nc", debug, reps, no_collective, phases, skip_in_dma, serial)
    if key in _NC_CACHE:
        return _NC_CACHE[key]
    nc = bacc.Bacc("TRN2", target_bir_lowering=False, debug=False,
                   num_devices=N_CORES)
    mul = mybir.AluOpType.mult
    add = mybir.AluOpType.add

    xT = nc.dram_tensor("xT", [D, T], BF16, kind="ExternalInput").ap()
    wqkv = nc.dram_tensor("wqkv", [D, QKV], BF16, kind="ExternalInput").ap()
    wo = nc.dram_tensor("wo", [D, D], BF16, kind="ExternalInput").ap()
    cos_all = nc.dram_tensor("cos_all", [T, E + HD], BF16,
                             kind="ExternalInput").ap()
    sin_all = nc.dram_tensor("sin_all", [T, E + HD], BF16,
                             kind="ExternalInput").ap()
    maskmul = nc.dram_tensor("maskmul", [128, NT, 128], BF16,
                             kind="ExternalInput").ap()
    outT = nc.dram_tensor("outT", [TS, D], F32, kind="ExternalOutput").ap()
    # sink for the PE warm-keeper chains (never read by the host)
    warm_sink = nc.dram_tensor("warm_sink", [1, 2], F32,
                               kind="ExternalOutput").ap()
    dbg = {}
    if debug:
        for nm, shape in [("d_qv", [128, NT, 448]), ("d_roped", [128, NT, E + HD]),
                          ("d_qT", [128, 2, T]), ("d_kT", [128, T]),
                          ("d_exp00", [128, T]), ("d_exp03", [128, T]),
                          ("d_pot0", [128, T]), ("d_rcp0", [64, T]),
                          ("d_ot0", [64, T])]:
            dbg[nm] = nc.dram_tensor(nm, shape, F32, kind="ExternalOutput").ap()

    hview = lambda ap: ap.rearrange("p (h d) -> p h d", h=HQ + 1)

    with tile.TileContext(nc) as tc:
        with ExitStack() as top:
            persist = top.enter_context(tc.tile_pool(name="persist", bufs=1))
            dram = top.enter_context(tc.tile_pool(name="dram", bufs=1, space="DRAM"))

            # ---- one-time constants ----
            ident = persist.tile([128, 128], BF16)
            make_identity(nc, ident[:])
            eps_t = persist.tile([128, 1], F32)
            nc.gpsimd.memset(eps_t[:], EPS)
            # scratch for the PE warm-keeper chains (values irrelevant)
            warm_w = persist.tile([128, 128], BF16)
            nc.gpsimd.memset(warm_w[:], 0.0)
            warm_x = persist.tile([128, 512], BF16)
            nc.gpsimd.memset(warm_x[:], 0.0)
            warm_wf = persist.tile([128, 128], F32)
            nc.gpsimd.memset(warm_wf[:], 0.0)
            warm_xf = persist.tile([128, 512], F32)
            nc.gpsimd.memset(warm_xf[:], 0.0)

            # ---- persistent (per-rep-rewritten) tiles ----
            qT_all = persist.tile([128, 2, T], BF16, name="qT_all")
            kT_all = persist.tile([128, T], BF16, name="kT_all")
            # per-t-tile fused [ones 0:64 | v 64:128 | qk 128:448]; the
            # AV lhsT is cols 0:128, so the softmax denominator comes out
            # at psum partitions 0-63 (base 0 -- custom-DVE-op legal) and
            # O^T at 64-127
            qv = [persist.tile([128, 448], BF16, name=f"qv{j}")
                  for j in range(NT)]
            cs_c = persist.tile([128, NT, E + HD], BF16, name="cs_c")
            cs_s = persist.tile([128, NT, E + HD], BF16, name="cs_s")
            mask_sb = persist.tile([128, NT, 128], BF16, name="mask_sb")
            wo_sb = persist.tile([128, NK, D], BF16, name="wo_sb")
            xt_g = [persist.tile([128, 2, T], BF16, name=f"xt{g}")
                    for g in range(NK // 2)]
            wq_g = [persist.tile([128, 2, QKV], BF16, name=f"wq{g}")
                    for g in range(NK // 2)]

            xT_v = xT.rearrange("(k p) t -> p k t", p=128)
            wq_v = wqkv.rearrange("(k p) n -> p k n", p=128)
            wo_v = wo.rearrange("(k p) n -> p k n", p=128)

            cos_v = cos_all.rearrange("(j p) n -> p j n", p=128)
            sin_v = sin_all.rearrange("(j p) n -> p j n", p=128)

            def load_inputs():
                ins = []
                # rope tables for the first two t-tiles up front (tiny);
                # the rest after x/w so the projection isn't DMA-starved
                ins.append(nc.scalar.dma_start(cs_c[:, 0:2, :],
                                               cos_v[:, 0:2, :]))
                ins.append(nc.scalar.dma_start(cs_s[:, 0:2, :],
                                               sin_v[:, 0:2, :]))
                # x^T and W_qkv^T split per 2-k-tile group, alternating
                # the two HWDGE queues; x^T additionally split into t-halves
                # -- t-tiles 0-3 of the projection need only cols 0:512 of
                # every contraction chunk, so their psum groups (and the
                # norm/rope chains behind them) finish ~1.3MB of DMA earlier
                for g in range(NK // 2):
                    eng = nc.sync if g % 2 == 0 else nc.scalar
                    ins.append(eng.dma_start(xt_g[g][:, :, 0:512],
                                             xT_v[:, 2 * g:2 * g + 2, 0:512]))
                    ins.append(eng.dma_start(wq_g[g][:], wq_v[:, 2 * g:2 * g + 2, :]))
                ins.append(nc.scalar.dma_start(cs_c[:, 2:4, :],
                                               cos_v[:, 2:4, :]))
                ins.append(nc.scalar.dma_start(cs_s[:, 2:4, :],
                                               sin_v[:, 2:4, :]))
                for g in range(NK // 2):
                    eng = nc.sync if g % 2 == 0 else nc.scalar
                    ins.append(eng.dma_start(xt_g[g][:, :, 512:T],
                                             xT_v[:, 2 * g:2 * g + 2, 512:T]))
                ins.append(nc.scalar.dma_start(cs_c[:, 4:NT, :],
                                               cos_v[:, 4:NT, :]))
                ins.append(nc.scalar.dma_start(cs_s[:, 4:NT, :],
                                               sin_v[:, 4:NT, :]))
                ins.append(nc.scalar.dma_start(mask_sb[:], maskmul[:]))
                # 8MB Wo load in the background, after x/w on both queues
                for q in range(8):
                    eng = nc.sync if q % 2 == 0 else nc.scalar
                    ins.append(eng.dma_start(
                        wo_sb[:, 2 * q:2 * (q + 1), :],
                        wo_v[:, 2 * q:2 * (q + 1), :]))
                return ins

            prev_tail = None
            for _rep in range(reps):
                entries = load_inputs()
                # ones columns for the AV denominator rows
                for j in range(NT):
                    nc.gpsimd.memset(qv[j][:, 0:64], 1.0)

                # ---- phase A: QKV projection + norm + rope + transposes ----
                pend_tr = []  # transposes lagged one t-tile to keep PE dense

                def do_transposes(j, roped):
                    for m in range(2):
                        ptr = ps_tr.tile([128, 128], BF16, name="ptr", tag="ptr")
                        nc.tensor.transpose(
                            ptr[:], roped[:, 128 * m:128 * (m + 1)], ident[:])
                        nc.scalar.copy(qT_all[:, m, 128 * j:128 * (j + 1)], ptr[:])
                    ptrk = ps_tr.tile([64, 128], BF16, name="ptrk", tag="ptrk")
                    nc.tensor.transpose(ptrk[:], roped[:, E:E + HD], ident[:])
                    nc.scalar.copy(kT_all[0:HD, 128 * j:128 * (j + 1)], ptrk[:])
                    # per-tile dup into partitions 64..127 for odd-head MMs
                    return nc.sync.dma_start(
                        kT_all[HD:128, 128 * j:128 * (j + 1)],
                        kT_all[0:HD, 128 * j:128 * (j + 1)])

                # A-1: QKV projection, k-outer over 8 concurrent psum
                # groups -- each matmul needs only contraction chunk k, so
                # the PE paces with the x/w DMA stream instead of
                # head-of-line blocking on the last chunk.
                with ExitStack() as pa0:
                    ps_qkv = pa0.enter_context(
                        tc.tile_pool(name="ps_qkv", bufs=1, space="PSUM"))
                    pqs = [ps_qkv.tile([128, QKV], F32, name=f"pq{j}",
                                       tag=f"pq{j}") for j in range(NT)]
                    # two passes: t-tiles 0-3 (x cols 0:512) fully
                    # accumulate and evict while the second x half streams
                    for (j0, j1) in ((0, NT // 2), (NT // 2, NT)):
                        for k in range(NK):
                            for j in range(j0, j1):
                                nc.tensor.matmul(
                                    pqs[j][:],
                                    xt_g[k // 2][:, k % 2,
                                                 128 * j:128 * (j + 1)],
                                    wq_g[k // 2][:, k % 2, :],
                                    start=(k == 0), stop=(k == NK - 1))
                        for j in range(j0, j1):
                            # single psum eviction: Q|K for rope + V slice
                            # (DVE: the Act queue paces the A-tail)
                            nc.vector.tensor_copy(qv[j][:, 64:448], pqs[j][:])
                            if debug:
                                nc.sync.dma_start(dbg["d_qv"][:, j, :],
                                                  qv[j][:].bitcast(F32))

                # A-2: norm + rope + transposes (psum freed by A-1)
                with ExitStack() as pa:
                    scrA = pa.enter_context(tc.tile_pool(name="scrA", bufs=4))
                    ps_tr = pa.enter_context(
                        tc.tile_pool(name="ps_tr", bufs=2, space="PSUM"))

                    stats = {}

                    def do_stats(j):
                        # sum of squares per head for t-tile j (DVE)
                        qk = qv[j][:, 128:448]
                        sq = scrA.tile([128, E + HD], BF16, name="sq", tag="sq")
                        nc.vector.tensor_tensor(out=sq[:], in0=qk, in1=qk, op=mul)
                        ssq = scrA.tile([128, 8], F32, name="ssq", tag="ssq")
                        nc.vector.tensor_reduce(
                            out=ssq[:, 0:HQ + 1],
                            in_=sq[:].rearrange("p (h d) -> p h d", h=HQ + 1),
                            axis=mybir.AxisListType.X, op=add)
                        stats[j] = ssq

                    rmss = {}

                    def do_sqrt(j):
                        ssq = stats.pop(j)
                        rms = scrA.tile([128, 8], F32, name="rms", tag="rms")
                        nc.scalar.activation(rms[:, 0:HQ + 1], ssq[:, 0:HQ + 1],
                                             mybir.ActivationFunctionType.Sqrt,
                                             scale=1.0 / HD, bias=eps_t[:])
                        rmss[j] = rms

                    def do_rope(j):
                        # inv scale then rope (DVE/Pool)
                        qk = qv[j][:, 128:448]
                        rms = rmss.pop(j)
                        inv = scrA.tile([128, 8], F32, name="inv", tag="inv")
                        nc.vector.reciprocal(inv[:, 0:HQ + 1], rms[:, 0:HQ + 1])
                        # rope: qs = qk * inv[head]; m1 = qs*cos (gpsimd);
                        # m2 = swap(qs)*sin (gpsimd); roped = m1 + m2
                        qs = scrA.tile([128, E + HD], BF16, name="qs", tag="qs")
                        m1 = scrA.tile([128, E + HD], BF16, name="m1", tag="m1")
                        m2 = scrA.tile([128, E + HD], BF16, name="m2", tag="m2")
                        roped = scrA.tile([128, E + HD], BF16, name="roped",
                                          tag="roped")
                        inv_b = (inv[:, 0:HQ + 1]
                                 .rearrange("p (h o) -> p h o", o=1)
                                 .broadcast_to([128, HQ + 1, HD]))
                        nc.vector.tensor_tensor(out=hview(qs[:]),
                                                in0=hview(qk),
                                                in1=inv_b, op=mul)
                        nc.vector.tensor_tensor(
                            out=hview(m1[:]), in0=hview(qs[:]),
                            in1=hview(cs_c[:, j, :]), op=mul)
                        nc.gpsimd.tensor_tensor(
                            out=hview(m2[:])[:, :, 0:32],
                            in0=hview(qs[:])[:, :, 32:HD],
                            in1=hview(cs_s[:, j, :])[:, :, 0:32], op=mul)
                        nc.gpsimd.tensor_tensor(
                            out=hview(m2[:])[:, :, 32:HD],
                            in0=hview(qs[:])[:, :, 0:32],
                            in1=hview(cs_s[:, j, :])[:, :, 32:HD], op=mul)
                        nc.vector.tensor_tensor(out=roped[:], in0=m1[:],
                                                in1=m2[:], op=add)
                        if debug:
                            nc.sync.dma_start(dbg["d_roped"][:, j, :],
                                              roped[:].bitcast(F32))
                        pend_tr.append((j, roped))

                    # software pipeline: sqrt lags stats by one tile, rope
                    # by two, transposes by three, so no engine queue blocks
                    # on work another engine hasn't finished yet
                    for j in range(NT):
                        do_stats(j)
                        if j >= 1:
                            do_sqrt(j - 1)
                        if j >= 2:
                            do_rope(j - 2)
                        if j >= 3:
                            do_transposes(*pend_tr.pop(0))
                    do_sqrt(NT - 1)
                    do_rope(NT - 2)
                    do_transposes(*pend_tr.pop(0))
                    do_rope(NT - 1)
                    while pend_tr:
                        tail = do_transposes(*pend_tr.pop(0))

                def fin():
                    nonlocal prev_tail
                    if serial and prev_tail is not None:
                        for e in entries:
                            add_dep_helper(e.ins, prev_tail.ins, sync=True,
                                           reason="serialize reps")
                    prev_tail = tail
                if debug:
                    nc.sync.dma_start(dbg["d_qT"][:], qT_all[:].bitcast(F32))
                    nc.sync.dma_start(dbg["d_kT"][:], kT_all[:].bitcast(F32))

                if "B" not in phases:
                    fin()
                    continue

                # ---- phase B: attention, head pairs in PE row groups ----
                a2a_in = [dram.tile([N_CORES, 2 * HD, TS], BF16,
                                    name=f"a2i{p}_{_rep}", tag=f"a2i{p}_{_rep}")
                          for p in range(2)]
                a2a_out = [dram.tile([N_CORES, 2 * HD, TS], BF16,
                                     name=f"a2o{p}_{_rep}", tag=f"a2o{p}_{_rep}")
                           for p in range(2)]
                colls = []
                with ExitStack() as pb:
                    epool = pb.enter_context(tc.tile_pool(name="expp", bufs=4))
                    bscr = pb.enter_context(tc.tile_pool(name="scrB", bufs=2))
                    ps_st = pb.enter_context(
                        tc.tile_pool(name="ps_st", bufs=4, space="PSUM"))
                    ps_ot = pb.enter_context(
                        tc.tile_pool(name="ps_ot", bufs=2, space="PSUM"))

                    for p in range(2):
                        pots = [ps_ot.tile([128, T], F32, name=f"pot{p}{hh}",
                                           tag="pot") for hh in range(2)]
                        ots = [bscr.tile([128, T], BF16, name=f"ot{hh}",
                                         tag="ot") for hh in range(2)]

                        def do_tail(a, b, hop_eng):
                            # normalize + stage O^T for psum cols [a, b):
                            # rcp of den (psum-direct at base 0) -> partition
                            # hop of the small rcp tile -> multiply. The hop
                            # engine is sync for the mid-pair chunk (a scalar
                            # -queue DMA would block later exps on the Act
                            # SEQ) and scalar for the end-of-pair chunk.
                            wdt = b - a
                            for hh in range(2):
                                rcp = bscr.tile([HD, 512], F32,
                                                name="rcp", tag="rcp")
                                nc.vector.reciprocal_approx_fast(
                                    out=rcp[:, 0:wdt], in_=pots[hh][0:HD, a:b])
                                rcp_hi = bscr.tile([128, 512], F32,
                                                   name="rcphi", tag="rcphi")
                                hop_eng.dma_start(rcp_hi[HD:128, 0:wdt],
                                                  rcp[:, 0:wdt])
                                nc.vector.tensor_tensor(
                                    out=ots[hh][HD:128, a:b],
                                    in0=pots[hh][HD:128, a:b],
                                    in1=rcp_hi[HD:128, 0:wdt], op=mul)
                                j0, j1 = a // TS, b // TS
                                nc.sync.dma_start(
                                    a2a_in[p][j0:j1, HD * hh:HD * (hh + 1), :]
                                    .rearrange("j d t -> d j t"),
                                    ots[hh][HD:128, a:b].rearrange(
                                        "d (j t) -> d j t", j=j1 - j0))

                        def do_av(i, ets):
                            t0 = 128 * i
                            for hh in range(2):
                                for (a, b) in ((0, 512), (512, 1024)):
                                    if t0 >= b:
                                        continue
                                    lo = max(a, t0)
                                    nc.tensor.matmul(
                                        pots[hh][:, lo:b],
                                        qv[i][:, 0:128],
                                        ets[hh][:, lo - t0:b - t0],
                                        start=(i == 0),
                                        stop=(i == min(b // 128, NT) - 1))

                        # AV matmuls lag the score matmuls by one s-tile so
                        # the PE never queues behind an exp it must wait for
                        pend_av = []
                        for i in range(NT):
                            t0 = 128 * i
                            w = T - t0
                            ets = [epool.tile([128, T], BF16, name=f"et{hh}",
                                              tag="et") for hh in range(2)]
                            # paired score matmuls in different PE row
                            # groups, 512-col psum chunks (1 bank each) so
                            # the pipeline holds 4 outstanding chunks
                            for (c0, cw) in chunks_for(w):
                                psts = [ps_st.tile([128, 512], F32,
                                                   name="pst", tag="pst")
                                        for hh in range(2)]
                                for hh in range(2):
                                    p0 = 64 * hh
                                    nc.tensor.matmul(
                                        psts[hh][:, 0:cw],
                                        kT_all[p0:p0 + HD, t0:t0 + 128],
                                        qT_all[p0:p0 + HD, p,
                                               t0 + c0:t0 + c0 + cw],
                                        start=True, stop=True)
                                for hh in range(2):
                                    nc.scalar.activation(
                                        ets[hh][:, c0:c0 + cw],
                                        psts[hh][:, 0:cw],
                                        mybir.ActivationFunctionType.Exp,
                                        scale=float(1.0 / np.sqrt(HD)))
                                    if c0 == 0:
                                        # causal mask on the diagonal block
                                        meng = nc.gpsimd if p == 0 else nc.vector
                                        meng.tensor_tensor(
                                            out=ets[hh][:, 0:128],
                                            in0=ets[hh][:, 0:128],
                                            in1=mask_sb[:, i, :], op=mul)
                            pend_av.append((i, ets))
                            if len(pend_av) > 1:
                                do_av(*pend_av.pop(0))
                                if i == 4:
                                    # psum cols 0-511 complete after AV(3):
                                    # run half the normalize tail mid-pair
                                    do_tail(0, 512, nc.sync)
                        do_av(*pend_av.pop(0))
                        do_tail(512, 1024, nc.scalar)
                        if debug and p == 0:
                            dpot = bscr.tile([128, T], F32, name="dpot", tag="dpot")
                            nc.vector.tensor_copy(dpot[:], pots[0][:])
                            nc.sync.dma_start(dbg["d_pot0"][:], dpot[:])
                            nc.sync.dma_start(dbg["d_ot0"][:],
                                              ots[0][HD:128, :].bitcast(F32))
                        if no_collective:
                            colls.append(nc.sync.dma_start(a2a_out[p][:],
                                                           a2a_in[p][:]))
                        else:
                            colls.append(nc.gpsimd.collective_compute(
                                "AllToAll", mybir.AluOpType.bypass,
                                replica_groups=[list(range(N_CORES))],
                                ins=[a2a_in[p].opt()], outs=[a2a_out[p].opt()]))
                tail = colls[-1]

                # ---- phase C: token-parallel out-projection ----
                # contraction row 256*src + 64*h + d: piece 0 (heads 0-1)
                # supplies even k-tiles, piece 1 (heads 2-3) odd k-tiles.
                if "C" not in phases:
                    fin()
                    continue
                with ExitStack() as pc:
                    agp = pc.enter_context(tc.tile_pool(name="agp", bufs=1))
                    oscr = pc.enter_context(tc.tile_pool(name="oscr", bufs=1))
                    ps_o = pc.enter_context(
                        tc.tile_pool(name="ps_o", bufs=1, space="PSUM"))
                    ps_w = pc.enter_context(
                        tc.tile_pool(name="ps_w", bufs=1, space="PSUM"))

                    def keep_warm(n, idx, fp32):
                        # dep-free matmuls bridge the PE-idle gap of an
                        # in-flight collective so the next real matmuls run
                        # at full clock (HAM stays engaged); sunk to DRAM so
                        # the chain can't be pruned
                        pw = ps_w.tile([128, 512], F32, name=f"pw{idx}",
                                       tag="pw")
                        ww, wx = (warm_wf, warm_xf) if fp32 else (warm_w, warm_x)
                        for k in range(n):
                            nc.tensor.matmul(pw[:], ww[:], wx[:],
                                             start=(k == 0), stop=(k == n - 1))
                        wsb = oscr.tile([1, 2], F32, name=f"wsb{idx}",
                                        tag=f"wsb{idx}")
                        nc.scalar.copy(wsb[:], pw[0:1, 0:2])
                        nc.sync.dma_start(warm_sink[:], wsb[:])

                    pos = [ps_o.tile([128, 512], F32, name=f"po{c}",
                                     tag=f"po{c}") for c in range(4)]
                    for p in range(2):
                        keep_warm(*((12, 0, False) if p == 0 else (16, 1, True)))
                        agt = agp.tile([128, NT, TS], BF16, name=f"agt{p}",
                                       tag=f"agt{p}")
                        nc.scalar.dma_start(
                            agt[:], a2a_out[p].rearrange("s p t -> p s t"))
                        for s in range(NT):
                            for c in range(4):
                                nc.tensor.matmul(
                                    pos[c],
                                    agt[:, s, :],
                                    wo_sb[:, 2 * s + p, 512 * c:512 * (c + 1)],
                                    start=(p == 0 and s == 0),
                                    stop=(p == 1 and s == NT - 1))
                    out_sb = oscr.tile([128, 4, 512], F32, name="out_sb",
                                       tag="out_sb")
                    for c in range(4):
                        if c % 2 == 0:
                            nc.scalar.copy(out_sb[:, c, :], pos[c][:])
                        else:
                            nc.vector.tensor_copy(out_sb[:, c, :], pos[c][:])
                        tail = nc.sync.dma_start(
                            outT[:, 512 * c:512 * (c + 1)], out_sb[:, c, :])
                fin()

    nc.compile()
    _NC_CACHE[key] = nc
    return nc


def prep_in_maps(inputs):
    """Host-side sharding + layout prep. Returns per-core input maps."""
    x = np.asarray(inputs["x"], dtype=np.float32)
    mask = np.asarray(inputs["mask"])
    cos = np.asarray(inputs["cos"], dtype=np.float32)
    sin = np.asarray(inputs["sin"], dtype=np.float32)
    Wq = np.asarray(inputs["Wq"], dtype=np.float32)
    Wk = np.asarray(inputs["Wk"], dtype=np.float32)
    Wv = np.asarray(inputs["Wv"], dtype=np.float32)
    Wo = np.asarray(inputs["Wo"], dtype=np.float32)
    qw = np.asarray(inputs["q_norm_w"], dtype=np.float32)
    kw = np.asarray(inputs["k_norm_w"], dtype=np.float32)

    bf = ml_dtypes.bfloat16
    xT = np.ascontiguousarray(x.T).astype(bf)

    # norm weights folded into per-(q/k) rope tables; sin table carries the
    # rotate-half signs: out[i] = w[i]*(q[i]*cos[i] + rot[i]*sin[i]),
    # rot[i] = -q[i+32] (i<32) else q[i-32]
    sgn = np.concatenate([-np.ones(HD // 2, np.float32),
                          np.ones(HD // 2, np.float32)])
    cos_q = cos * qw[None, :]
    sin_q = sin * (sgn * qw)[None, :]
    cos_k = cos * kw[None, :]
    sin_k = sin * (sgn * kw)[None, :]
    cos_all = np.ascontiguousarray(
        np.concatenate([cos_q] * HQ + [cos_k], axis=1)).astype(bf)
    sin_all = np.ascontiguousarray(
        np.concatenate([sin_q] * HQ + [sin_k], axis=1)).astype(bf)

    # multiplicative mask for diagonal blocks, in S^T layout:
    # maskmul[s', i, t'] = 0 where mask[128i+t', 128i+s'] else 1
    mm = np.ones((128, NT, 128), np.float32)
    for i in range(NT):
        blk = mask[128 * i:128 * (i + 1), 128 * i:128 * (i + 1)]
        mm[:, i, :] = np.where(blk.T, 0.0, 1.0).astype(np.float32)
    maskmul = mm.astype(bf)

    # token-parallel out-projection: full Wo^T on every core; its
    # contraction row order 256*src + 64*h + d matches the AllToAll output
    woT = np.ascontiguousarray(Wo.T).astype(bf)                # [2048, 2048]
    in_maps = []
    for r in range(N_CORES):
        wq_r = Wq[E * r:E * (r + 1), :]          # [256, 2048]
        wk_r = Wk[HD * r:HD * (r + 1), :]        # [64, 2048]
        wv_r = Wv[HD * r:HD * (r + 1), :]        # [64, 2048]
        # V first: the projection psum is [V | Q | K], evicted to qv cols
        # 64:448 behind the ones block
        wqkv = np.ascontiguousarray(
            np.concatenate([wv_r, wq_r, wk_r], axis=0).T).astype(bf)
        in_maps.append({
            "xT": xT, "wqkv": wqkv, "wo": woT,
            "cos_all": cos_all, "sin_all": sin_all,
            "maskmul": maskmul,
        })
    return in_maps


def kernel(**inputs) -> np.ndarray:
    nc = build_nc()
    in_maps = prep_in_maps(inputs)
    res = run_bass_kernel_spmd(nc, in_maps, list(range(N_CORES)))
    out = np.empty((T, D), dtype=np.float32)
    for r in range(N_CORES):
        out[TS * r:TS * (r + 1), :] = res.results[r]["outT"]
    return out


# revision 71
# speedup vs baseline: 1.0132x; 1.0009x over previous
"""Trainium2 Bass kernel for GroupQueryAttention (T=1024, D=2048, H=32, KV=8, HD=64).

Sharding: tensor-parallel over heads across 8 NeuronCores. Core r owns q-heads
4r..4r+3 and kv-head r (split Wq/Wk/Wv output dims). x is replicated. The
per-core attention outputs are exchanged with an AllToAll (split in two
pieces, overlapped with compute) and the out-projection is token-parallel:
each core computes its 128-token slice of the final [1024, 2048] output with
the full 2048-deep contraction.

Per-core dataflow:
  A) fused QKV projection run k-outer over 8 concurrent psum groups so the
     PE paces with the x/w DMA stream (inputs split per 2-k-tile chunk over
     both HWDGE queues); then a software-pipelined RMSNorm + RoPE chain in
     bf16 on DVE/GpSimd (norm weight folded into host-side cos/sin tables,
     stats -> sqrt -> rope -> PE-transpose each lagged one t-tile).
  B) attention in transposed layout, heads processed in pairs (h even at
     partitions 0-63, h odd at 64-127) so the K=64 score matmuls of the two
     heads run concurrently in different PE row groups; AV matmuls lag the
     scores by one s-tile so the PE never queues behind an exp. The V tile
     carries 64 ones-columns, so the AV matmul emits O^T (psum rows 0-63)
     and the softmax denominator replicated 64x (rows 64-127) in one
     stream; the normalize tail (den copy, partition-hop DMA, approx
     reciprocal, multiply) runs per 512-col chunk -- the first half
     mid-pair, off the critical path.
  C) two-piece AllToAll (heads 0-1 staged right after pair 1 and overlapped
     with pair-2 compute; heads 2-3 after). The out-projection accumulates
     piece-1 k-tiles (even) while piece 2 is on the wire. Dep-free
     warm-keeper matmuls on scratch data bridge the PE-idle collective
     gaps so the out-projection runs at full clock (HAM stays engaged).
"""
import sys
import numpy as np
import ml_dtypes

sys.path.insert(0, "/opt/trn_rl_repo")

import concourse.bass as bass
import concourse.mybir as mybir
import concourse.tile as tile
import concourse.bacc as bacc
from concourse.bass_utils import run_bass_kernel_spmd
from concourse.masks import make_identity
from concourse.tile_rust import add_dep_helper
from contextlib import ExitStack

N_CORES = 8
T, D, H, KVH, HD = 1024, 2048, 32, 8, 64
GROUP = H // KVH          # 4 q-heads per kv-head
HQ = H // N_CORES         # 4 q-heads per core
E = HQ * HD               # 256 = per-core q width
QKV = E + HD + HD         # 384 = fused projection width
NT = T // 128             # 8 t-tiles
NK = D // 128             # 16 contraction tiles
EPS = 1e-6
F32 = mybir.dt.float32
BF16 = mybir.dt.bfloat16
TS = T // N_CORES         # 128 = per-core token slice of the output

_NC_CACHE = {}


def chunks_for(width):
    out = []
    for a in range(0, width, 512):
        out.append((a, min(width - a, 512)))
    return out


def build_nc(debug=False, reps=1, no_collective=False, phases="ABC",
             skip_in_dma=False, serial=False):
    key = ("nc", debug, reps, no_collective, phases, skip_in_dma, serial)
    if key in _NC_CACHE:
        return _NC_CACHE[key]
    nc = bacc.Bacc("TRN2", target_bir_lowering=False, debug=False,
                   num_devices=N_CORES)
    mul = mybir.AluOpType.mult
    add = mybir.AluOpType.add

    xT = nc.dram_tensor("xT", [D, T], BF16, kind="ExternalInput").ap()
    wqkv = nc.dram_tensor("wqkv", [D, QKV], BF16, kind="ExternalInput").ap()
    wo = nc.dram_tensor("wo", [D, D], BF16, kind="ExternalInput").ap()
    cos_all = nc.dram_tensor("cos_all", [T, E + HD], BF16,
                             kind="ExternalInput").ap()
    sin_all = nc.dram_tensor("sin_all", [T, E + HD], BF16,
                             kind="ExternalInput").ap()
    maskmul = nc.dram_tensor("maskmul", [128, NT, 128], BF16,
                             kind="ExternalInput").ap()
    outT = nc.dram_tensor("outT", [TS, D], F32, kind="ExternalOutput").ap()
    # sink for the PE warm-keeper chains (never read by the host)
    warm_sink = nc.dram_tensor("warm_sink", [1, 2], F32,
                               kind="ExternalOutput").ap()
    dbg = {}
    if debug:
        for nm, shape in [("d_qv", [128, NT, 448]), ("d_roped", [128, NT, E + HD]),
                          ("d_qT", [128, 2, T]), ("d_kT", [128, T]),
                          ("d_exp00", [128, T]), ("d_exp03", [128, T]),
                          ("d_pot0", [128, T]), ("d_rcp0", [64, T]),
                          ("d_ot0", [64, T])]:
            dbg[nm] = nc.dram_tensor(nm, shape, F32, kind="ExternalOutput").ap()

    hview = lambda ap: ap.rearrange("p (h d) -> p h d", h=HQ + 1)

    with tile.TileContext(nc) as tc:
        with ExitStack() as top:
            persist = top.enter_context(tc.tile_pool(name="persist", bufs=1))
            dram = top.enter_context(tc.tile_pool(name="dram", bufs=1, space="DRAM"))

            # ---- one-time constants ----
            ident = persist.tile([128, 128], BF16)
            make_identity(nc, ident[:])
            eps_t = persist.tile([128, 1], F32)
            nc.gpsimd.memset(eps_t[:], EPS)
            # scratch for the PE warm-keeper chains (values irrelevant)
            warm_w = persist.tile([128, 128], BF16)
            nc.gpsimd.memset(warm_w[:], 0.0)
            warm_x = persist.tile([128, 512], BF16)
            nc.gpsimd.memset(warm_x[:], 0.0)
            warm_wf = persist.tile([128, 128], F32)
            nc.gpsimd.memset(warm_wf[:], 0.0)
            warm_xf = persist.tile([128, 512], F32)
            nc.gpsimd.memset(warm_xf[:], 0.0)

            # ---- persistent (per-rep-rewritten) tiles ----
            qT_all = persist.tile([128, 2, T], BF16, name="qT_all")
            kT_all = persist.tile([128, T], BF16, name="kT_all")
            # per-t-tile fused [ones 0:64 | v 64:128 | qk 128:448]; the
            # AV lhsT is cols 0:128, so the softmax denominator comes out
            # at psum partitions 0-63 (base 0 -- custom-DVE-op legal) and
            # O^T at 64-127
            qv = [persist.tile([128, 448], BF16, name=f"qv{j}")
                  for j in range(NT)]
            cs_c = persist.tile([128, NT, E + HD], BF16, name="cs_c")
            cs_s = persist.tile([128, NT, E + HD], BF16, name="cs_s")
            mask_sb = persist.tile([128, NT, 128], BF16, name="mask_sb")
            wo_sb = persist.tile([128, NK, D], BF16, name="wo_sb")
            xt_g = [persist.tile([128, 2, T], BF16, name=f"xt{g}")
                    for g in range(NK // 2)]
            wq_g = [persist.tile([128, 2, QKV], BF16, name=f"wq{g}")
                    for g in range(NK // 2)]

            xT_v = xT.rearrange("(k p) t -> p k t", p=128)
            wq_v = wqkv.rearrange("(k p) n -> p k n", p=128)
            wo_v = wo.rearrange("(k p) n -> p k n", p=128)

            cos_v = cos_all.rearrange("(j p) n -> p j n", p=128)
            sin_v = sin_all.rearrange("(j p) n -> p j n", p=128)

            def load_inputs():
                ins = []
                # rope tables for the first two t-tiles up front (tiny);
                # the rest after x/w so the projection isn't DMA-starved
                ins.append(nc.scalar.dma_start(cs_c[:, 0:2, :],
                                               cos_v[:, 0:2, :]))
                ins.append(nc.scalar.dma_start(cs_s[:, 0:2, :],
                                               sin_v[:, 0:2, :]))
                # x^T and W_qkv^T split per 2-k-tile group, alternating
                # the two HWDGE queues; x^T additionally split into t-halves
                # -- t-tiles 0-3 of the projection need only cols 0:512 of
                # every contraction chunk, so their psum groups (and the
                # norm/rope chains behind them) finish ~1.3MB of DMA earlier
                for g in range(NK // 2):
                    eng = nc.sync if g % 2 == 0 else nc.scalar
                    ins.append(eng.dma_start(xt_g[g][:, :, 0:512],
                                             xT_v[:, 2 * g:2 * g + 2, 0:512]))
                    ins.append(eng.dma_start(wq_g[g][:], wq_v[:, 2 * g:2 * g + 2, :]))
                ins.append(nc.scalar.dma_start(cs_c[:, 2:4, :],
                                               cos_v[:, 2:4, :]))
                ins.append(nc.scalar.dma_start(cs_s[:, 2:4, :],
                                               sin_v[:, 2:4, :]))
                for g in range(NK // 2):
                    eng = nc.sync if g % 2 == 0 else nc.scalar
                    ins.append(eng.dma_start(xt_g[g][:, :, 512:T],
                                             xT_v[:, 2 * g:2 * g + 2, 512:T]))
                ins.append(nc.scalar.dma_start(cs_c[:, 4:NT, :],
                                               cos_v[:, 4:NT, :]))
                ins.append(nc.scalar.dma_start(cs_s[:, 4:NT, :],
                                               sin_v[:, 4:NT, :]))
                ins.append(nc.scalar.dma_start(mask_sb[:], maskmul[:]))
                # 8MB Wo load in the background, after x/w on both queues
                for q in range(8):
                    eng = nc.sync if q % 2 == 0 else nc.scalar
                    ins.append(eng.dma_start(
                        wo_sb[:, 2 * q:2 * (q + 1), :],
                        wo_v[:, 2 * q:2 * (q + 1), :]))
                return ins

            prev_tail = None
            for _rep in range(reps):
                entries = load_inputs()
                # ones columns for the AV denominator rows
                for j in range(NT):
                    nc.gpsimd.memset(qv[j][:, 0:64], 1.0)

                # ---- phase A: QKV projection + norm + rope + transposes ----
                pend_tr = []  # transposes lagged one t-tile to keep PE dense

                def do_transposes(j, roped):
                    for m in range(2):
                        ptr = ps_tr.tile([128, 128], BF16, name="ptr", tag="ptr")
                        nc.tensor.transpose(
                            ptr[:], roped[:, 128 * m:128 * (m + 1)], ident[:])
                        nc.scalar.copy(qT_all[:, m, 128 * j:128 * (j + 1)], ptr[:])
                    ptrk = ps_tr.tile([64, 128], BF16, name="ptrk", tag="ptrk")
                    nc.tensor.transpose(ptrk[:], roped[:, E:E + HD], ident[:])
                    nc.scalar.copy(kT_all[0:HD, 128 * j:128 * (j + 1)], ptrk[:])
                    # per-tile dup into partitions 64..127 for odd-head MMs
                    return nc.sync.dma_start(
                        kT_all[HD:128, 128 * j:128 * (j + 1)],
                        kT_all[0:HD, 128 * j:128 * (j + 1)])

                # A-1: QKV projection, k-outer over 8 concurrent psum
                # groups -- each matmul needs only contraction chunk k, so
                # the PE paces with the x/w DMA stream instead of
                # head-of-line blocking on the last chunk.
                with ExitStack() as pa0:
                    ps_qkv = pa0.enter_context(
                        tc.tile_pool(name="ps_qkv", bufs=1, space="PSUM"))
                    pqs = [ps_qkv.tile([128, QKV], F32, name=f"pq{j}",
                                       tag=f"pq{j}") for j in range(NT)]
                    # two passes: t-tiles 0-3 (x cols 0:512) fully
                    # accumulate and evict while the second x half streams
                    for (j0, j1) in ((0, NT // 2), (NT // 2, NT)):
                        for k in range(NK):
                            for j in range(j0, j1):
                                nc.tensor.matmul(
                                    pqs[j][:],
                                    xt_g[k // 2][:, k % 2,
                                                 128 * j:128 * (j + 1)],
                                    wq_g[k // 2][:, k % 2, :],
                                    start=(k == 0), stop=(k == NK - 1))
                        for j in range(j0, j1):
                            # single psum eviction: Q|K for rope + V slice
                            # (DVE: the Act queue paces the A-tail)
                            nc.vector.tensor_copy(qv[j][:, 64:448], pqs[j][:])
                            if debug:
                                nc.sync.dma_start(dbg["d_qv"][:, j, :],
                                                  qv[j][:].bitcast(F32))

                # A-2: norm + rope + transposes (psum freed by A-1)
                with ExitStack() as pa:
                    scrA = pa.enter_context(tc.tile_pool(name="scrA", bufs=5))
                    ps_tr = pa.enter_context(
                        tc.tile_pool(name="ps_tr", bufs=2, space="PSUM"))

                    stats = {}

                    def do_stats(j):
                        # sum of squares per head for t-tile j (DVE)
                        qk = qv[j][:, 128:448]
                        sq = scrA.tile([128, E + HD], BF16, name="sq", tag="sq")
                        nc.vector.tensor_tensor(out=sq[:], in0=qk, in1=qk, op=mul)
                        ssq = scrA.tile([128, 8], F32, name="ssq", tag="ssq")
                        nc.vector.tensor_reduce(
                            out=ssq[:, 0:HQ + 1],
                            in_=sq[:].rearrange("p (h d) -> p h d", h=HQ + 1),
                            axis=mybir.AxisListType.X, op=add)
                        stats[j] = ssq

                    rmss = {}

                    def do_sqrt(j):
                        ssq = stats.pop(j)
                        rms = scrA.tile([128, 8], F32, name="rms", tag="rms")
                        nc.scalar.activation(rms[:, 0:HQ + 1], ssq[:, 0:HQ + 1],
                                             mybir.ActivationFunctionType.Sqrt,
                                             scale=1.0 / HD, bias=eps_t[:])
                        rmss[j] = rms

                    def do_rope(j):
                        # inv scale then rope (DVE/Pool)
                        qk = qv[j][:, 128:448]
                        rms = rmss.pop(j)
                        inv = scrA.tile([128, 8], F32, name="inv", tag="inv")
                        nc.vector.reciprocal(inv[:, 0:HQ + 1], rms[:, 0:HQ + 1])
                        # rope: qs = qk * inv[head]; m1 = qs*cos (gpsimd);
                        # m2 = swap(qs)*sin (gpsimd); roped = m1 + m2
                        qs = scrA.tile([128, E + HD], BF16, name="qs", tag="qs")
                        m1 = scrA.tile([128, E + HD], BF16, name="m1", tag="m1")
                        m2 = scrA.tile([128, E + HD], BF16, name="m2", tag="m2")
                        roped = scrA.tile([128, E + HD], BF16, name="roped",
                                          tag="roped")
                        inv_b = (inv[:, 0:HQ + 1]
                                 .rearrange("p (h o) -> p h o", o=1)
                                 .broadcast_to([128, HQ + 1, HD]))
                        nc.vector.tensor_tensor(out=hview(qs[:]),
                                                in0=hview(qk),
                                                in1=inv_b, op=mul)
                        nc.vector.tensor_tensor(
                            out=hview(m1[:]), in0=hview(qs[:]),
                            in1=hview(cs_c[:, j, :]), op=mul)
                        nc.gpsimd.tensor_tensor(
                            out=hview(m2[:])[:, :, 0:32],
                            in0=hview(qs[:])[:, :, 32:HD],
                            in1=hview(cs_s[:, j, :])[:, :, 0:32], op=mul)
                        nc.gpsimd.tensor_tensor(
                            out=hview(m2[:])[:, :, 32:HD],
                            in0=hview(qs[:])[:, :, 0:32],
                            in1=hview(cs_s[:, j, :])[:, :, 32:HD], op=mul)
                        nc.vector.tensor_tensor(out=roped[:], in0=m1[:],
                                                in1=m2[:], op=add)
                        if debug:
                            nc.sync.dma_start(dbg["d_roped"][:, j, :],
                                              roped[:].bitcast(F32))
                        pend_tr.append((j, roped))

                    # software pipeline: sqrt lags stats by one tile, rope
                    # by two, transposes by three, so no engine queue blocks
                    # on work another engine hasn't finished yet
                    for j in range(NT):
                        do_stats(j)
                        if j >= 1:
                            do_sqrt(j - 1)
                        if j >= 2:
                            do_rope(j - 2)
                        if j >= 3:
                            do_transposes(*pend_tr.pop(0))
                    do_sqrt(NT - 1)
                    do_rope(NT - 2)
                    do_transposes(*pend_tr.pop(0))
                    do_rope(NT - 1)
                    while pend_tr:
                        tail = do_transposes(*pend_tr.pop(0))

                def fin():
                    nonlocal prev_tail
                    if serial and prev_tail is not None:
                        for e in entries:
                            add_dep_helper(e.ins, prev_tail.ins, sync=True,
                                           reason="serialize reps")
                    prev_tail = tail
                if debug:
                    nc.sync.dma_start(dbg["d_qT"][:], qT_all[:].bitcast(F32))
                    nc.sync.dma_start(dbg["d_kT"][:], kT_all[:].bitcast(F32))

                if "B" not in phases:
                    fin()
                    continue

                # ---- phase B: attention, head pairs in PE row groups ----
                a2a_in = [dram.tile([N_CORES, 2 * HD, TS], BF16,
                                    name=f"a2i{p}_{_rep}", tag=f"a2i{p}_{_rep}")
                          for p in range(2)]
                a2a_out = [dram.tile([N_CORES, 2 * HD, TS], BF16,
                                     name=f"a2o{p}_{_rep}", tag=f"a2o{p}_{_rep}")
                           for p in range(2)]
                colls = []
                with ExitStack() as pb:
                    epool = pb.enter_context(tc.tile_pool(name="expp", bufs=4))
                    bscr = pb.enter_context(tc.tile_pool(name="scrB", bufs=2))
                    ps_st = pb.enter_context(
                        tc.tile_pool(name="ps_st", bufs=4, space="PSUM"))
                    ps_ot = pb.enter_context(
                        tc.tile_pool(name="ps_ot", bufs=2, space="PSUM"))

                    for p in range(2):
                        pots = [ps_ot.tile([128, T], F32, name=f"pot{p}{hh}",
                                           tag="pot") for hh in range(2)]
                        ots = [bscr.tile([128, T], BF16, name=f"ot{hh}",
                                         tag="ot") for hh in range(2)]

                        def do_tail(a, b, hop_eng):
                            # normalize + stage O^T for psum cols [a, b):
                            # rcp of den (psum-direct at base 0) -> partition
                            # hop of the small rcp tile -> multiply. The hop
                            # engine is sync for the mid-pair chunk (a scalar
                            # -queue DMA would block later exps on the Act
                            # SEQ) and scalar for the end-of-pair chunk.
                            wdt = b - a
                            for hh in range(2):
                                rcp = bscr.tile([HD, 512], F32,
                                                name="rcp", tag="rcp")
                                nc.vector.reciprocal_approx_fast(
                                    out=rcp[:, 0:wdt], in_=pots[hh][0:HD, a:b])
                                rcp_hi = bscr.tile([128, 512], F32,
                                                   name="rcphi", tag="rcphi")
                                hop_eng.dma_start(rcp_hi[HD:128, 0:wdt],
                                                  rcp[:, 0:wdt])
                                nc.vector.tensor_tensor(
                                    out=ots[hh][HD:128, a:b],
                                    in0=pots[hh][HD:128, a:b],
                                    in1=rcp_hi[HD:128, 0:wdt], op=mul)
                                j0, j1 = a // TS, b // TS
                                nc.sync.dma_start(
                                    a2a_in[p][j0:j1, HD * hh:HD * (hh + 1), :]
                                    .rearrange("j d t -> d j t"),
                                    ots[hh][HD:128, a:b].rearrange(
                                        "d (j t) -> d j t", j=j1 - j0))

                        def do_av(i, ets):
                            t0 = 128 * i
                            for hh in range(2):
                                for (a, b) in ((0, 512), (512, 1024)):
                                    if t0 >= b:
                                        continue
                                    lo = max(a, t0)
                                    nc.tensor.matmul(
                                        pots[hh][:, lo:b],
                                        qv[i][:, 0:128],
                                        ets[hh][:, lo - t0:b - t0],
                                        start=(i == 0),
                                        stop=(i == min(b // 128, NT) - 1))

                        # AV matmuls lag the score matmuls by one s-tile so
                        # the PE never queues behind an exp it must wait for
                        pend_av = []
                        for i in range(NT):
                            t0 = 128 * i
                            w = T - t0
                            ets = [epool.tile([128, T], BF16, name=f"et{hh}",
                                              tag="et") for hh in range(2)]
                            # paired score matmuls in different PE row
                            # groups, 512-col psum chunks (1 bank each) so
                            # the pipeline holds 4 outstanding chunks
                            for (c0, cw) in chunks_for(w):
                                psts = [ps_st.tile([128, 512], F32,
                                                   name="pst", tag="pst")
                                        for hh in range(2)]
                                for hh in range(2):
                                    p0 = 64 * hh
                                    nc.tensor.matmul(
                                        psts[hh][:, 0:cw],
                                        kT_all[p0:p0 + HD, t0:t0 + 128],
                                        qT_all[p0:p0 + HD, p,
                                               t0 + c0:t0 + c0 + cw],
                                        start=True, stop=True)
                                for hh in range(2):
                                    nc.scalar.activation(
                                        ets[hh][:, c0:c0 + cw],
                                        psts[hh][:, 0:cw],
                                        mybir.ActivationFunctionType.Exp,
                                        scale=float(1.0 / np.sqrt(HD)))
                                    if c0 == 0:
                                        # causal mask on the diagonal block
                                        meng = nc.gpsimd if p == 0 else nc.vector
                                        meng.tensor_tensor(
                                            out=ets[hh][:, 0:128],
                                            in0=ets[hh][:, 0:128],
                                            in1=mask_sb[:, i, :], op=mul)
                            pend_av.append((i, ets))
                            if len(pend_av) > 1:
                                do_av(*pend_av.pop(0))
                                if i == 4:
                                    # psum cols 0-511 complete after AV(3):
                                    # run half the normalize tail mid-pair
                                    do_tail(0, 512, nc.sync)
                        do_av(*pend_av.pop(0))
                        do_tail(512, 1024, nc.scalar)
                        if debug and p == 0:
                            dpot = bscr.tile([128, T], F32, name="dpot", tag="dpot")
                            nc.vector.tensor_copy(dpot[:], pots[0][:])
                            nc.sync.dma_start(dbg["d_pot0"][:], dpot[:])
                            nc.sync.dma_start(dbg["d_ot0"][:],
                                              ots[0][HD:128, :].bitcast(F32))
                        if no_collective:
                            colls.append(nc.sync.dma_start(a2a_out[p][:],
                                                           a2a_in[p][:]))
                        else:
                            colls.append(nc.gpsimd.collective_compute(
                                "AllToAll", mybir.AluOpType.bypass,
                                replica_groups=[list(range(N_CORES))],
                                ins=[a2a_in[p].opt()], outs=[a2a_out[p].opt()]))
                tail = colls[-1]

                # ---- phase C: token-parallel out-projection ----
                # contraction row 256*src + 64*h + d: piece 0 (heads 0-1)
                # supplies even k-tiles, piece 1 (heads 2-3) odd k-tiles.
                if "C" not in phases:
                    fin()
                    continue
                with ExitStack() as pc:
                    agp = pc.enter_context(tc.tile_pool(name="agp", bufs=1))
                    oscr = pc.enter_context(tc.tile_pool(name="oscr", bufs=1))
                    ps_o = pc.enter_context(
                        tc.tile_pool(name="ps_o", bufs=1, space="PSUM"))
                    ps_w = pc.enter_context(
                        tc.tile_pool(name="ps_w", bufs=1, space="PSUM"))

                    def keep_warm(n, idx, fp32):
                        # dep-free matmuls bridge the PE-idle gap of an
                        # in-flight collective so the next real matmuls run
                        # at full clock (HAM stays engaged); sunk to DRAM so
                        # the chain can't be pruned
                        pw = ps_w.tile([128, 512], F32, name=f"pw{idx}",
                                       tag="pw")
                        ww, wx = (warm_wf, warm_xf) if fp32 else (warm_w, warm_x)
                        for k in range(n):
                            nc.tensor.matmul(pw[:], ww[:], wx[:],
                                             start=(k == 0), stop=(k == n - 1))
                        wsb = oscr.tile([1, 2], F32, name=f"wsb{idx}",
                                        tag=f"wsb{idx}")
                        nc.scalar.copy(wsb[:], pw[0:1, 0:2])
                        nc.sync.dma_start(warm_sink[:], wsb[:])

                    pos = [ps_o.tile([128, 512], F32, name=f"po{c}",
                                     tag=f"po{c}") for c in range(4)]
                    for p in range(2):
                        keep_warm(*((12, 0, False) if p == 0 else (16, 1, True)))
                        agt = agp.tile([128, NT, TS], BF16, name=f"agt{p}",
                                       tag=f"agt{p}")
                        nc.scalar.dma_start(
                            agt[:], a2a_out[p].rearrange("s p t -> p s t"))
                        for s in range(NT):
                            for c in range(4):
                                nc.tensor.matmul(
                                    pos[c],
                                    agt[:, s, :],
                                    wo_sb[:, 2 * s + p, 512 * c:512 * (c + 1)],
                                    start=(p == 0 and s == 0),
                                    stop=(p == 1 and s == NT - 1))
                    out_sb = oscr.tile([128, 4, 512], F32, name="out_sb",
                                       tag="out_sb")
                    for c in range(4):
                        if c % 2 == 0:
                            nc.scalar.copy(out_sb[:, c, :], pos[c][:])
                        else:
                            nc.vector.tensor_copy(out_sb[:, c, :], pos[c][:])
                        tail = nc.sync.dma_start(
                            outT[:, 512 * c:512 * (c + 1)], out_sb[:, c, :])
                fin()

    nc.compile()
    _NC_CACHE[key] = nc
    return nc


def prep_in_maps(inputs):
    """Host-side sharding + layout prep. Returns per-core input maps."""
    x = np.asarray(inputs["x"], dtype=np.float32)
    mask = np.asarray(inputs["mask"])
    cos = np.asarray(inputs["cos"], dtype=np.float32)
    sin = np.asarray(inputs["sin"], dtype=np.float32)
    Wq = np.asarray(inputs["Wq"], dtype=np.float32)
    Wk = np.asarray(inputs["Wk"], dtype=np.float32)
    Wv = np.asarray(inputs["Wv"], dtype=np.float32)
    Wo = np.asarray(inputs["Wo"], dtype=np.float32)
    qw = np.asarray(inputs["q_norm_w"], dtype=np.float32)
    kw = np.asarray(inputs["k_norm_w"], dtype=np.float32)

    bf = ml_dtypes.bfloat16
    xT = np.ascontiguousarray(x.T).astype(bf)

    # norm weights folded into per-(q/k) rope tables; sin table carries the
    # rotate-half signs: out[i] = w[i]*(q[i]*cos[i] + rot[i]*sin[i]),
    # rot[i] = -q[i+32] (i<32) else q[i-32]
    sgn = np.concatenate([-np.ones(HD // 2, np.float32),
                          np.ones(HD // 2, np.float32)])
    cos_q = cos * qw[None, :]
    sin_q = sin * (sgn * qw)[None, :]
    cos_k = cos * kw[None, :]
    sin_k = sin * (sgn * kw)[None, :]
    cos_all = np.ascontiguousarray(
        np.concatenate([cos_q] * HQ + [cos_k], axis=1)).astype(bf)
    sin_all = np.ascontiguousarray(
        np.concatenate([sin_q] * HQ + [sin_k], axis=1)).astype(bf)

    # multiplicative mask for diagonal blocks, in S^T layout:
    # maskmul[s', i, t'] = 0 where mask[128i+t', 128i+s'] else 1
    mm = np.ones((128, NT, 128), np.float32)
    for i in range(NT):
        blk = mask[128 * i:128 * (i + 1), 128 * i:128 * (i + 1)]
        mm[:, i, :] = np.where(blk.T, 0.0, 1.0).astype(np.float32)
    maskmul = mm.astype(bf)

    # token-parallel out-projection: full Wo^T on every core; its
    # contraction row order 256*src + 64*h + d matches the AllToAll output
    woT = np.ascontiguousarray(Wo.T).astype(bf)                # [2048, 2048]
    in_maps = []
    for r in range(N_CORES):
        wq_r = Wq[E * r:E * (r + 1), :]          # [256, 2048]
        wk_r = Wk[HD * r:HD * (r + 1), :]        # [64, 2048]
        wv_r = Wv[HD * r:HD * (r + 1), :]        # [64, 2048]
        # V first: the projection psum is [V | Q | K], evicted to qv cols
        # 64:448 behind the ones block
        wqkv = np.ascontiguousarray(
            np.concatenate([wv_r, wq_r, wk_r], axis=0).T).astype(bf)
        in_maps.append({
            "xT": xT, "wqkv": wqkv, "wo": woT,
            "cos_all": cos_all, "sin_all": sin_all,
            "maskmul": maskmul,
        })
    return in_maps


def kernel(**inputs) -> np.ndarray:
    nc = build_nc()
    in_maps = prep_in_maps(inputs)
    res = run_bass_kernel_spmd(nc, in_maps, list(range(N_CORES)))
    out = np.empty((T, D), dtype=np.float32)
    for r in range(N_CORES):
        out[TS * r:TS * (r + 1), :] = res.results[r]["outT"]
    return out


# revision 74
# speedup vs baseline: 1.0172x; 1.0040x over previous
"""Trainium2 Bass kernel for GroupQueryAttention (T=1024, D=2048, H=32, KV=8, HD=64).

Sharding: tensor-parallel over heads across 8 NeuronCores. Core r owns q-heads
4r..4r+3 and kv-head r (split Wq/Wk/Wv output dims). x is replicated. The
per-core attention outputs are exchanged with an AllToAll (split in two
pieces, overlapped with compute) and the out-projection is token-parallel:
each core computes its 128-token slice of the final [1024, 2048] output with
the full 2048-deep contraction.

Per-core dataflow:
  A) fused QKV projection run k-outer over 8 concurrent psum groups so the
     PE paces with the x/w DMA stream (inputs split per 2-k-tile chunk over
     both HWDGE queues); then a software-pipelined RMSNorm + RoPE chain in
     bf16 on DVE/GpSimd (norm weight folded into host-side cos/sin tables,
     stats -> sqrt -> rope -> PE-transpose each lagged one t-tile).
  B) attention in transposed layout, heads processed in pairs (h even at
     partitions 0-63, h odd at 64-127) so the K=64 score matmuls of the two
     heads run concurrently in different PE row groups; AV matmuls lag the
     scores by one s-tile so the PE never queues behind an exp. The V tile
     carries 64 ones-columns, so the AV matmul emits O^T (psum rows 0-63)
     and the softmax denominator replicated 64x (rows 64-127) in one
     stream; the normalize tail (den copy, partition-hop DMA, approx
     reciprocal, multiply) runs per 512-col chunk -- the first half
     mid-pair, off the critical path.
  C) two-piece AllToAll (heads 0-1 staged right after pair 1 and overlapped
     with pair-2 compute; heads 2-3 after). The out-projection accumulates
     piece-1 k-tiles (even) while piece 2 is on the wire. Dep-free
     warm-keeper matmuls on scratch data bridge the PE-idle collective
     gaps so the out-projection runs at full clock (HAM stays engaged).
"""
import sys
import numpy as np
import ml_dtypes

sys.path.insert(0, "/opt/trn_rl_repo")

import concourse.bass as bass
import concourse.mybir as mybir
import concourse.tile as tile
import concourse.bacc as bacc
from concourse.bass_utils import run_bass_kernel_spmd
from concourse.masks import make_identity
from concourse.tile_rust import add_dep_helper
from contextlib import ExitStack

N_CORES = 8
T, D, H, KVH, HD = 1024, 2048, 32, 8, 64
GROUP = H // KVH          # 4 q-heads per kv-head
HQ = H // N_CORES         # 4 q-heads per core
E = HQ * HD               # 256 = per-core q width
QKV = E + HD + HD         # 384 = fused projection width
NT = T // 128             # 8 t-tiles
NK = D // 128             # 16 contraction tiles
EPS = 1e-6
F32 = mybir.dt.float32
BF16 = mybir.dt.bfloat16
TS = T // N_CORES         # 128 = per-core token slice of the output

_NC_CACHE = {}


def chunks_for(width):
    out = []
    for a in range(0, width, 512):
        out.append((a, min(width - a, 512)))
    return out


def build_nc(debug=False, reps=1, no_collective=False, phases="ABC",
             skip_in_dma=False, serial=False):
    key = ("nc", debug, reps, no_collective, phases, skip_in_dma, serial)
    if key in _NC_CACHE:
        return _NC_CACHE[key]
    nc = bacc.Bacc("TRN2", target_bir_lowering=False, debug=False,
                   num_devices=N_CORES)
    mul = mybir.AluOpType.mult
    add = mybir.AluOpType.add

    xT = nc.dram_tensor("xT", [D, T], BF16, kind="ExternalInput").ap()
    wqkv = nc.dram_tensor("wqkv", [D, QKV], BF16, kind="ExternalInput").ap()
    wo = nc.dram_tensor("wo", [D, D], BF16, kind="ExternalInput").ap()
    cos_all = nc.dram_tensor("cos_all", [T, E + HD], BF16,
                             kind="ExternalInput").ap()
    sin_all = nc.dram_tensor("sin_all", [T, E + HD], BF16,
                             kind="ExternalInput").ap()
    maskmul = nc.dram_tensor("maskmul", [128, NT, 128], BF16,
                             kind="ExternalInput").ap()
    outT = nc.dram_tensor("outT", [TS, D], F32, kind="ExternalOutput").ap()
    # sink for the PE warm-keeper chains (never read by the host)
    warm_sink = nc.dram_tensor("warm_sink", [1, 2], F32,
                               kind="ExternalOutput").ap()
    dbg = {}
    if debug:
        for nm, shape in [("d_qv", [128, NT, 448]), ("d_roped", [128, NT, E + HD]),
                          ("d_qT", [128, 2, T]), ("d_kT", [128, T]),
                          ("d_exp00", [128, T]), ("d_exp03", [128, T]),
                          ("d_pot0", [128, T]), ("d_rcp0", [64, T]),
                          ("d_ot0", [64, T])]:
            dbg[nm] = nc.dram_tensor(nm, shape, F32, kind="ExternalOutput").ap()

    hview = lambda ap: ap.rearrange("p (h d) -> p h d", h=HQ + 1)

    with tile.TileContext(nc) as tc:
        with ExitStack() as top:
            persist = top.enter_context(tc.tile_pool(name="persist", bufs=1))
            dram = top.enter_context(tc.tile_pool(name="dram", bufs=1, space="DRAM"))

            # ---- one-time constants ----
            ident = persist.tile([128, 128], BF16)
            make_identity(nc, ident[:])
            eps_t = persist.tile([128, 1], F32)
            nc.gpsimd.memset(eps_t[:], EPS)
            # scratch for the PE warm-keeper chains (values irrelevant)
            warm_w = persist.tile([128, 128], BF16)
            nc.gpsimd.memset(warm_w[:], 0.0)
            warm_x = persist.tile([128, 512], BF16)
            nc.gpsimd.memset(warm_x[:], 0.0)
            warm_wf = persist.tile([128, 128], F32)
            nc.gpsimd.memset(warm_wf[:], 0.0)
            warm_xf = persist.tile([128, 512], F32)
            nc.gpsimd.memset(warm_xf[:], 0.0)

            # ---- persistent (per-rep-rewritten) tiles ----
            qT_all = persist.tile([128, 2, T], BF16, name="qT_all")
            kT_all = persist.tile([128, T], BF16, name="kT_all")
            # per-t-tile fused [ones 0:64 | v 64:128 | qk 128:448]; the
            # AV lhsT is cols 0:128, so the softmax denominator comes out
            # at psum partitions 0-63 (base 0 -- custom-DVE-op legal) and
            # O^T at 64-127
            qv = [persist.tile([128, 448], BF16, name=f"qv{j}")
                  for j in range(NT)]
            cs_c = persist.tile([128, NT, E + HD], BF16, name="cs_c")
            cs_s = persist.tile([128, NT, E + HD], BF16, name="cs_s")
            mask_sb = persist.tile([128, NT, 128], BF16, name="mask_sb")
            wo_sb = persist.tile([128, NK, D], BF16, name="wo_sb")
            xt_g = [persist.tile([128, 2, T], BF16, name=f"xt{g}")
                    for g in range(NK // 2)]
            wq_g = [persist.tile([128, 2, QKV], BF16, name=f"wq{g}")
                    for g in range(NK // 2)]

            xT_v = xT.rearrange("(k p) t -> p k t", p=128)
            wq_v = wqkv.rearrange("(k p) n -> p k n", p=128)
            wo_v = wo.rearrange("(k p) n -> p k n", p=128)

            cos_v = cos_all.rearrange("(j p) n -> p j n", p=128)
            sin_v = sin_all.rearrange("(j p) n -> p j n", p=128)

            def load_inputs():
                ins = []
                # rope tables for the first two t-tiles up front (tiny);
                # the rest after x/w so the projection isn't DMA-starved
                ins.append(nc.scalar.dma_start(cs_c[:, 0:2, :],
                                               cos_v[:, 0:2, :]))
                ins.append(nc.scalar.dma_start(cs_s[:, 0:2, :],
                                               sin_v[:, 0:2, :]))
                # x^T and W_qkv^T split per 2-k-tile group, alternating
                # the two HWDGE queues; x^T additionally split into t-halves
                # -- t-tiles 0-3 of the projection need only cols 0:512 of
                # every contraction chunk, so their psum groups (and the
                # norm/rope chains behind them) finish ~1.3MB of DMA earlier
                for g in range(NK // 2):
                    eng = nc.sync if g % 2 == 0 else nc.scalar
                    ins.append(eng.dma_start(xt_g[g][:, :, 0:512],
                                             xT_v[:, 2 * g:2 * g + 2, 0:512]))
                    ins.append(eng.dma_start(wq_g[g][:], wq_v[:, 2 * g:2 * g + 2, :]))
                ins.append(nc.scalar.dma_start(cs_c[:, 2:4, :],
                                               cos_v[:, 2:4, :]))
                ins.append(nc.scalar.dma_start(cs_s[:, 2:4, :],
                                               sin_v[:, 2:4, :]))
                for g in range(NK // 2):
                    eng = nc.sync if g % 2 == 0 else nc.scalar
                    ins.append(eng.dma_start(xt_g[g][:, :, 512:T],
                                             xT_v[:, 2 * g:2 * g + 2, 512:T]))
                ins.append(nc.scalar.dma_start(cs_c[:, 4:NT, :],
                                               cos_v[:, 4:NT, :]))
                ins.append(nc.scalar.dma_start(cs_s[:, 4:NT, :],
                                               sin_v[:, 4:NT, :]))
                ins.append(nc.scalar.dma_start(mask_sb[:], maskmul[:]))
                # 8MB Wo load in the background, after x/w on both queues
                for q in range(8):
                    eng = nc.sync if q % 2 == 0 else nc.scalar
                    ins.append(eng.dma_start(
                        wo_sb[:, 2 * q:2 * (q + 1), :],
                        wo_v[:, 2 * q:2 * (q + 1), :]))
                return ins

            prev_tail = None
            for _rep in range(reps):
                entries = load_inputs()
                # ones columns for the AV denominator rows
                for j in range(NT):
                    nc.gpsimd.memset(qv[j][:, 0:64], 1.0)

                # ---- phase A: QKV projection + norm + rope + transposes ----
                pend_tr = []  # transposes lagged one t-tile to keep PE dense

                def do_transposes(j, roped):
                    for m in range(2):
                        ptr = ps_tr.tile([128, 128], BF16, name="ptr", tag="ptr")
                        nc.tensor.transpose(
                            ptr[:], roped[:, 128 * m:128 * (m + 1)], ident[:])
                        nc.scalar.copy(qT_all[:, m, 128 * j:128 * (j + 1)], ptr[:])
                    ptrk = ps_tr.tile([64, 128], BF16, name="ptrk", tag="ptrk")
                    nc.tensor.transpose(ptrk[:], roped[:, E:E + HD], ident[:])
                    nc.scalar.copy(kT_all[0:HD, 128 * j:128 * (j + 1)], ptrk[:])
                    # per-tile dup into partitions 64..127 for odd-head MMs
                    return nc.sync.dma_start(
                        kT_all[HD:128, 128 * j:128 * (j + 1)],
                        kT_all[0:HD, 128 * j:128 * (j + 1)])

                # A-1: QKV projection, k-outer over 8 concurrent psum
                # groups -- each matmul needs only contraction chunk k, so
                # the PE paces with the x/w DMA stream instead of
                # head-of-line blocking on the last chunk.
                with ExitStack() as pa0:
                    ps_qkv = pa0.enter_context(
                        tc.tile_pool(name="ps_qkv", bufs=1, space="PSUM"))
                    pqs = [ps_qkv.tile([128, QKV], F32, name=f"pq{j}",
                                       tag=f"pq{j}") for j in range(NT)]
                    # two passes: t-tiles 0-3 (x cols 0:512) fully
                    # accumulate and evict while the second x half streams
                    for (j0, j1) in ((0, NT // 2), (NT // 2, NT)):
                        for k in range(NK):
                            for j in range(j0, j1):
                                nc.tensor.matmul(
                                    pqs[j][:],
                                    xt_g[k // 2][:, k % 2,
                                                 128 * j:128 * (j + 1)],
                                    wq_g[k // 2][:, k % 2, :],
                                    start=(k == 0), stop=(k == NK - 1))
                        for j in range(j0, j1):
                            # single psum eviction: Q|K for rope + V slice
                            # (DVE: the Act queue paces the A-tail)
                            nc.vector.tensor_copy(qv[j][:, 64:448], pqs[j][:])
                            if debug:
                                nc.sync.dma_start(dbg["d_qv"][:, j, :],
                                                  qv[j][:].bitcast(F32))

                # A-2: norm + rope + transposes (psum freed by A-1)
                with ExitStack() as pa:
                    scrA = pa.enter_context(tc.tile_pool(name="scrA", bufs=5))
                    ps_tr = pa.enter_context(
                        tc.tile_pool(name="ps_tr", bufs=2, space="PSUM"))

                    stats = {}

                    def do_stats(j):
                        # sum of squares per head for t-tile j (DVE)
                        qk = qv[j][:, 128:448]
                        sq = scrA.tile([128, E + HD], BF16, name="sq", tag="sq")
                        nc.vector.tensor_tensor(out=sq[:], in0=qk, in1=qk, op=mul)
                        ssq = scrA.tile([128, 8], F32, name="ssq", tag="ssq")
                        nc.vector.tensor_reduce(
                            out=ssq[:, 0:HQ + 1],
                            in_=sq[:].rearrange("p (h d) -> p h d", h=HQ + 1),
                            axis=mybir.AxisListType.X, op=add)
                        stats[j] = ssq

                    rmss = {}

                    def do_sqrt(j):
                        ssq = stats.pop(j)
                        rms = scrA.tile([128, 8], F32, name="rms", tag="rms")
                        nc.scalar.activation(rms[:, 0:HQ + 1], ssq[:, 0:HQ + 1],
                                             mybir.ActivationFunctionType.Sqrt,
                                             scale=1.0 / HD, bias=eps_t[:])
                        rmss[j] = rms

                    def do_rope(j):
                        # inv scale then rope (DVE/Pool)
                        qk = qv[j][:, 128:448]
                        rms = rmss.pop(j)
                        inv = scrA.tile([128, 8], F32, name="inv", tag="inv")
                        nc.vector.reciprocal(inv[:, 0:HQ + 1], rms[:, 0:HQ + 1])
                        # rope: qs = qk * inv[head]; m1 = qs*cos (gpsimd);
                        # m2 = swap(qs)*sin (gpsimd); roped = m1 + m2
                        qs = scrA.tile([128, E + HD], BF16, name="qs", tag="qs")
                        m1 = scrA.tile([128, E + HD], BF16, name="m1", tag="m1")
                        m2 = scrA.tile([128, E + HD], BF16, name="m2", tag="m2")
                        roped = scrA.tile([128, E + HD], BF16, name="roped",
                                          tag="roped")
                        inv_b = (inv[:, 0:HQ + 1]
                                 .rearrange("p (h o) -> p h o", o=1)
                                 .broadcast_to([128, HQ + 1, HD]))
                        nc.vector.tensor_tensor(out=hview(qs[:]),
                                                in0=hview(qk),
                                                in1=inv_b, op=mul)
                        nc.vector.tensor_tensor(
                            out=hview(m1[:]), in0=hview(qs[:]),
                            in1=hview(cs_c[:, j, :]), op=mul)
                        nc.gpsimd.tensor_tensor(
                            out=hview(m2[:])[:, :, 0:32],
                            in0=hview(qs[:])[:, :, 32:HD],
                            in1=hview(cs_s[:, j, :])[:, :, 0:32], op=mul)
                        nc.gpsimd.tensor_tensor(
                            out=hview(m2[:])[:, :, 32:HD],
                            in0=hview(qs[:])[:, :, 0:32],
                            in1=hview(cs_s[:, j, :])[:, :, 32:HD], op=mul)
                        nc.vector.tensor_tensor(out=roped[:], in0=m1[:],
                                                in1=m2[:], op=add)
                        if debug:
                            nc.sync.dma_start(dbg["d_roped"][:, j, :],
                                              roped[:].bitcast(F32))
                        pend_tr.append((j, roped))

                    # software pipeline: sqrt lags stats by one tile, rope
                    # by two, transposes by three, so no engine queue blocks
                    # on work another engine hasn't finished yet
                    for j in range(NT):
                        do_stats(j)
                        if j >= 1:
                            do_sqrt(j - 1)
                        if j >= 2:
                            do_rope(j - 2)
                        if j >= 3:
                            do_transposes(*pend_tr.pop(0))
                    do_sqrt(NT - 1)
                    do_rope(NT - 2)
                    do_transposes(*pend_tr.pop(0))
                    do_rope(NT - 1)
                    while pend_tr:
                        tail = do_transposes(*pend_tr.pop(0))

                def fin():
                    nonlocal prev_tail
                    if serial and prev_tail is not None:
                        for e in entries:
                            add_dep_helper(e.ins, prev_tail.ins, sync=True,
                                           reason="serialize reps")
                    prev_tail = tail
                if debug:
                    nc.sync.dma_start(dbg["d_qT"][:], qT_all[:].bitcast(F32))
                    nc.sync.dma_start(dbg["d_kT"][:], kT_all[:].bitcast(F32))

                if "B" not in phases:
                    fin()
                    continue

                # ---- phase B: attention, head pairs in PE row groups ----
                a2a_in = [dram.tile([N_CORES, 2 * HD, TS], BF16,
                                    name=f"a2i{p}_{_rep}", tag=f"a2i{p}_{_rep}")
                          for p in range(2)]
                a2a_out = [dram.tile([N_CORES, 2 * HD, TS], BF16,
                                     name=f"a2o{p}_{_rep}", tag=f"a2o{p}_{_rep}")
                           for p in range(2)]
                colls = []
                with ExitStack() as pb:
                    epool = pb.enter_context(tc.tile_pool(name="expp", bufs=4))
                    bscr = pb.enter_context(tc.tile_pool(name="scrB", bufs=2))
                    ps_st = pb.enter_context(
                        tc.tile_pool(name="ps_st", bufs=4, space="PSUM"))
                    ps_ot = pb.enter_context(
                        tc.tile_pool(name="ps_ot", bufs=2, space="PSUM"))

                    for p in range(2):
                        pots = [ps_ot.tile([128, T], F32, name=f"pot{p}{hh}",
                                           tag="pot") for hh in range(2)]
                        ots = [bscr.tile([128, T], BF16, name=f"ot{hh}",
                                         tag="ot") for hh in range(2)]

                        def do_tail(a, b, hop_eng):
                            # normalize + stage O^T for psum cols [a, b):
                            # rcp of den (psum-direct at base 0) -> partition
                            # hop of the small rcp tile -> multiply. The hop
                            # engine is sync for the mid-pair chunk (a scalar
                            # -queue DMA would block later exps on the Act
                            # SEQ) and scalar for the end-of-pair chunk.
                            wdt = b - a
                            for hh in range(2):
                                rcp = bscr.tile([HD, 512], F32,
                                                name="rcp", tag="rcp")
                                nc.vector.reciprocal_approx_fast(
                                    out=rcp[:, 0:wdt], in_=pots[hh][0:HD, a:b])
                                rcp_hi = bscr.tile([128, 512], F32,
                                                   name="rcphi", tag="rcphi")
                                hop_eng.dma_start(rcp_hi[HD:128, 0:wdt],
                                                  rcp[:, 0:wdt])
                                nc.vector.tensor_tensor(
                                    out=ots[hh][HD:128, a:b],
                                    in0=pots[hh][HD:128, a:b],
                                    in1=rcp_hi[HD:128, 0:wdt], op=mul)
                                j0, j1 = a // TS, b // TS
                                nc.sync.dma_start(
                                    a2a_in[p][j0:j1, HD * hh:HD * (hh + 1), :]
                                    .rearrange("j d t -> d j t"),
                                    ots[hh][HD:128, a:b].rearrange(
                                        "d (j t) -> d j t", j=j1 - j0))

                        def do_av(i, ets):
                            t0 = 128 * i
                            for hh in range(2):
                                for (a, b) in ((0, 512), (512, 1024)):
                                    if t0 >= b:
                                        continue
                                    lo = max(a, t0)
                                    nc.tensor.matmul(
                                        pots[hh][:, lo:b],
                                        qv[i][:, 0:128],
                                        ets[hh][:, lo - t0:b - t0],
                                        start=(i == 0),
                                        stop=(i == min(b // 128, NT) - 1))

                        # AV matmuls lag the score matmuls by one s-tile so
                        # the PE never queues behind an exp it must wait for
                        pend_av = []
                        for i in range(NT):
                            t0 = 128 * i
                            w = T - t0
                            ets = [epool.tile([128, T], BF16, name=f"et{hh}",
                                              tag="et") for hh in range(2)]
                            # paired score matmuls in different PE row
                            # groups, 512-col psum chunks (1 bank each) so
                            # the pipeline holds 4 outstanding chunks
                            for (c0, cw) in chunks_for(w):
                                psts = [ps_st.tile([128, 512], F32,
                                                   name="pst", tag="pst")
                                        for hh in range(2)]
                                for hh in range(2):
                                    p0 = 64 * hh
                                    nc.tensor.matmul(
                                        psts[hh][:, 0:cw],
                                        kT_all[p0:p0 + HD, t0:t0 + 128],
                                        qT_all[p0:p0 + HD, p,
                                               t0 + c0:t0 + c0 + cw],
                                        start=True, stop=True)
                                for hh in range(2):
                                    nc.scalar.activation(
                                        ets[hh][:, c0:c0 + cw],
                                        psts[hh][:, 0:cw],
                                        mybir.ActivationFunctionType.Exp,
                                        scale=float(1.0 / np.sqrt(HD)))
                                    if c0 == 0:
                                        # causal mask on the diagonal block
                                        meng = nc.vector
                                        meng.tensor_tensor(
                                            out=ets[hh][:, 0:128],
                                            in0=ets[hh][:, 0:128],
                                            in1=mask_sb[:, i, :], op=mul)
                            pend_av.append((i, ets))
                            if len(pend_av) > 1:
                                do_av(*pend_av.pop(0))
                                if i == 4:
                                    # psum cols 0-511 complete after AV(3):
                                    # run half the normalize tail mid-pair
                                    do_tail(0, 512, nc.sync)
                        do_av(*pend_av.pop(0))
                        do_tail(512, 1024, nc.scalar)
                        if debug and p == 0:
                            dpot = bscr.tile([128, T], F32, name="dpot", tag="dpot")
                            nc.vector.tensor_copy(dpot[:], pots[0][:])
                            nc.sync.dma_start(dbg["d_pot0"][:], dpot[:])
                            nc.sync.dma_start(dbg["d_ot0"][:],
                                              ots[0][HD:128, :].bitcast(F32))
                        if no_collective:
                            colls.append(nc.sync.dma_start(a2a_out[p][:],
                                                           a2a_in[p][:]))
                        else:
                            colls.append(nc.gpsimd.collective_compute(
                                "AllToAll", mybir.AluOpType.bypass,
                                replica_groups=[list(range(N_CORES))],
                                ins=[a2a_in[p].opt()], outs=[a2a_out[p].opt()]))
                tail = colls[-1]

                # ---- phase C: token-parallel out-projection ----
                # contraction row 256*src + 64*h + d: piece 0 (heads 0-1)
                # supplies even k-tiles, piece 1 (heads 2-3) odd k-tiles.
                if "C" not in phases:
                    fin()
                    continue
                with ExitStack() as pc:
                    agp = pc.enter_context(tc.tile_pool(name="agp", bufs=1))
                    oscr = pc.enter_context(tc.tile_pool(name="oscr", bufs=1))
                    ps_o = pc.enter_context(
                        tc.tile_pool(name="ps_o", bufs=1, space="PSUM"))
                    ps_w = pc.enter_context(
                        tc.tile_pool(name="ps_w", bufs=1, space="PSUM"))

                    def keep_warm(n, idx, fp32):
                        # dep-free matmuls bridge the PE-idle gap of an
                        # in-flight collective so the next real matmuls run
                        # at full clock (HAM stays engaged); sunk to DRAM so
                        # the chain can't be pruned
                        pw = ps_w.tile([128, 512], F32, name=f"pw{idx}",
                                       tag="pw")
                        ww, wx = (warm_wf, warm_xf) if fp32 else (warm_w, warm_x)
                        for k in range(n):
                            nc.tensor.matmul(pw[:], ww[:], wx[:],
                                             start=(k == 0), stop=(k == n - 1))
                        wsb = oscr.tile([1, 2], F32, name=f"wsb{idx}",
                                        tag=f"wsb{idx}")
                        nc.scalar.copy(wsb[:], pw[0:1, 0:2])
                        nc.sync.dma_start(warm_sink[:], wsb[:])

                    pos = [ps_o.tile([128, 512], F32, name=f"po{c}",
                                     tag=f"po{c}") for c in range(4)]
                    for p in range(2):
                        keep_warm(*((12, 0, False) if p == 0 else (16, 1, True)))
                        agt = agp.tile([128, NT, TS], BF16, name=f"agt{p}",
                                       tag=f"agt{p}")
                        nc.scalar.dma_start(
                            agt[:], a2a_out[p].rearrange("s p t -> p s t"))
                        for s in range(NT):
                            for c in range(4):
                                nc.tensor.matmul(
                                    pos[c],
                                    agt[:, s, :],
                                    wo_sb[:, 2 * s + p, 512 * c:512 * (c + 1)],
                                    start=(p == 0 and s == 0),
                                    stop=(p == 1 and s == NT - 1))
                    out_sb = oscr.tile([128, 4, 512], F32, name="out_sb",
                                       tag="out_sb")
                    for c in range(4):
                        if c % 2 == 0:
                            nc.scalar.copy(out_sb[:, c, :], pos[c][:])
                        else:
                            nc.vector.tensor_copy(out_sb[:, c, :], pos[c][:])
                        tail = nc.sync.dma_start(
                            outT[:, 512 * c:512 * (c + 1)], out_sb[:, c, :])
                fin()

    nc.compile()
    _NC_CACHE[key] = nc
    return nc


def prep_in_maps(inputs):
    """Host-side sharding + layout prep. Returns per-core input maps."""
    x = np.asarray(inputs["x"], dtype=np.float32)
    mask = np.asarray(inputs["mask"])
    cos = np.asarray(inputs["cos"], dtype=np.float32)
    sin = np.asarray(inputs["sin"], dtype=np.float32)
    Wq = np.asarray(inputs["Wq"], dtype=np.float32)
    Wk = np.asarray(inputs["Wk"], dtype=np.float32)
    Wv = np.asarray(inputs["Wv"], dtype=np.float32)
    Wo = np.asarray(inputs["Wo"], dtype=np.float32)
    qw = np.asarray(inputs["q_norm_w"], dtype=np.float32)
    kw = np.asarray(inputs["k_norm_w"], dtype=np.float32)

    bf = ml_dtypes.bfloat16
    xT = np.ascontiguousarray(x.T).astype(bf)

    # norm weights folded into per-(q/k) rope tables; sin table carries the
    # rotate-half signs: out[i] = w[i]*(q[i]*cos[i] + rot[i]*sin[i]),
    # rot[i] = -q[i+32] (i<32) else q[i-32]
    sgn = np.concatenate([-np.ones(HD // 2, np.float32),
                          np.ones(HD // 2, np.float32)])
    cos_q = cos * qw[None, :]
    sin_q = sin * (sgn * qw)[None, :]
    cos_k = cos * kw[None, :]
    sin_k = sin * (sgn * kw)[None, :]
    cos_all = np.ascontiguousarray(
        np.concatenate([cos_q] * HQ + [cos_k], axis=1)).astype(bf)
    sin_all = np.ascontiguousarray(
        np.concatenate([sin_q] * HQ + [sin_k], axis=1)).astype(bf)

    # multiplicative mask for diagonal blocks, in S^T layout:
    # maskmul[s', i, t'] = 0 where mask[128i+t', 128i+s'] else 1
    mm = np.ones((128, NT, 128), np.float32)
    for i in range(NT):
        blk = mask[128 * i:128 * (i + 1), 128 * i:128 * (i + 1)]
        mm[:, i, :] = np.where(blk.T, 0.0, 1.0).astype(np.float32)
    maskmul = mm.astype(bf)

    # token-parallel out-projection: full Wo^T on every core; its
    # contraction row order 256*src + 64*h + d matches the AllToAll output
    woT = np.ascontiguousarray(Wo.T).astype(bf)                # [2048, 2048]
    in_maps = []
    for r in range(N_CORES):
        wq_r = Wq[E * r:E * (r + 1), :]          # [256, 2048]
        wk_r = Wk[HD * r:HD * (r + 1), :]        # [64, 2048]
        wv_r = Wv[HD * r:HD * (r + 1), :]        # [64, 2048]
        # V first: the projection psum is [V | Q | K], evicted to qv cols
        # 64:448 behind the ones block
        wqkv = np.ascontiguousarray(
            np.concatenate([wv_r, wq_r, wk_r], axis=0).T).astype(bf)
        in_maps.append({
            "xT": xT, "wqkv": wqkv, "wo": woT,
            "cos_all": cos_all, "sin_all": sin_all,
            "maskmul": maskmul,
        })
    return in_maps


def kernel(**inputs) -> np.ndarray:
    nc = build_nc()
    in_maps = prep_in_maps(inputs)
    res = run_bass_kernel_spmd(nc, in_maps, list(range(N_CORES)))
    out = np.empty((T, D), dtype=np.float32)
    for r in range(N_CORES):
        out[TS * r:TS * (r + 1), :] = res.results[r]["outT"]
    return out
